# revision 12
# baseline (speedup 1.0000x reference)
"""DeepSets segment-reduce kernel for 8x Trainium2 NeuronCores.

Strategy (all shapes hardcoded for N=500000, C=H=128, O=64, NSEG=2048):
  - Transposed activation layout: features on SBUF partitions, nodes on the
    free axis, so segment reductions are free-axis operations.
  - Whole-segment sharding: every segment is assigned entirely to one core,
    round-robin by global sorted-width rank.  All 8 cores share an identical
    compile-time slot/tile geometry (SPMD-safe); per-core padding is small.
    No collective is needed - the host gather is the unshard.
  - Encoder BN is folded into the linear weights; each layer is relu(W'x+b').
  - bf16 activations/weights halve DMA bytes and SBUF traffic; PSUM stays
    fp32.  Error budget (2e-2) dwarfs bf16 noise.
  - Pad columns DUPLICATE the slot's first member, so the segment max is
    exact for free; the inflated segment sum is fixed by one rank-1
    subtraction per tile (npad * h3[first]).  Empty segments are patched on
    the host (their output row is the folded bias).
  - Per-slot segment sum AND max are single-instruction DVE tensor_scalar
    ops with accum_out (fold op = op1) reading post-relu bf16 h3 from SBUF.
  - Tiles are processed in pairs: one DMA per pair (2KB/partition lines),
    LDWEIGHTS amortized over both tiles, and relu3 is one ScalarE
    instruction over a 2-bank PSUM pair.
  - Final projection out = [sum|max|mean] @ Wo'.T + bo' runs per core on its
    own 256 segments; mean rides the sum through the Wmean block with a
    per-slot reciprocal row-scale.
"""

import os
import sys

import numpy as np

if "/opt/trn_rl_repo" not in sys.path:
    sys.path.insert(0, "/opt/trn_rl_repo")

import ml_dtypes

import concourse.bacc as bacc
import concourse.mybir as mybir
import concourse.tile as tile
from concourse import bass_utils

EPS = 1e-5
NSEG = 2048
NCORES = 8
C = 128
H = 128
O = 64
S = NSEG // NCORES  # segment slots per core (256)
MAX_TILE = 512  # PSUM bank limit (fp32)

_compiled_cache = {}


def _fold_bn(W, b, g, be, m, v):
    a = g / np.sqrt(v + EPS)
    Wp = W * a[:, None]
    bp = (b - m) * a + be
    return Wp.astype(np.float32), bp.astype(np.float32)


def _plan_tiles(slot_w):
    """Greedy-pack slots (widths descending) into tiles of <=MAX_TILE cols.

    Returns list of (slot_start, n_slots, padded_width, col_start) and the
    total padded column count.
    """
    tiles = []
    col = 0
    k = 0
    n = len(slot_w)
    while k < n:
        # multiples of 4 so bf16 half-slot views stay 4B-aligned (2x_1P)
        wt = (int(slot_w[k]) + 3) & ~3
        assert 0 < wt <= MAX_TILE, f"slot width {wt} unsupported"
        d = min(MAX_TILE // wt, n - k)
        tiles.append((k, d, wt, col))
        col += d * wt
        k += d
    return tiles, col


def _build_program(tiles, cols):
    """Emit the Bass/Tile program shared by all 8 cores."""
    nc = bacc.Bacc(
        "TRN2",
        target_bir_lowering=False,
        debug=False,
        num_devices=NCORES,
    )
    f32 = mybir.dt.float32
    bf16 = mybir.dt.bfloat16

    xT = nc.dram_tensor("xT", [C, cols], bf16, kind="ExternalInput").ap()
    w1 = nc.dram_tensor("w1", [C, H], bf16, kind="ExternalInput").ap()
    w2 = nc.dram_tensor("w2", [H, H], bf16, kind="ExternalInput").ap()
    w3 = nc.dram_tensor("w3", [H, H], bf16, kind="ExternalInput").ap()
    b1 = nc.dram_tensor("b1", [H, 1], f32, kind="ExternalInput").ap()
    b2 = nc.dram_tensor("b2", [H, 1], f32, kind="ExternalInput").ap()
    b3 = nc.dram_tensor("b3", [H, 1], f32, kind="ExternalInput").ap()
    npad = nc.dram_tensor("npad", [H, S], f32, kind="ExternalInput").ap()
    wsum = nc.dram_tensor("wsum", [H, O], f32, kind="ExternalInput").ap()
    wmax = nc.dram_tensor("wmax", [H, O], f32, kind="ExternalInput").ap()
    wmean = nc.dram_tensor("wmean", [H, O], f32, kind="ExternalInput").ap()
    bo = nc.dram_tensor("bo", [1, O], f32, kind="ExternalInput").ap()
    # column ch holds the reciprocals for segment chunk ch (128 slots each)
    recip = nc.dram_tensor("recip", [H, S // H], f32, kind="ExternalInput").ap()
    out = nc.dram_tensor("out", [S, O], f32, kind="ExternalOutput").ap()

    relu = mybir.ActivationFunctionType.Relu
    add = mybir.AluOpType.add
    amax = mybir.AluOpType.max
    asub = mybir.AluOpType.subtract
    amult = mybir.AluOpType.mult

    PAIR = 2 * MAX_TILE

    with tile.TileContext(nc) as tc:
        with (
            tc.tile_pool(name="const", bufs=1) as cpool,
            tc.tile_pool(name="xin", bufs=4) as xpool,
            tc.tile_pool(name="h1", bufs=4) as h1pool,
            tc.tile_pool(name="h2", bufs=4) as h2pool,
            tc.tile_pool(name="h3", bufs=3) as h3pool,
            tc.tile_pool(name="scr", bufs=2) as scrpool,
            tc.tile_pool(name="acc", bufs=1) as accpool,
            tc.tile_pool(name="ps1", bufs=2, space="PSUM") as ps1,
            tc.tile_pool(name="ps2", bufs=2, space="PSUM") as ps2,
            tc.tile_pool(name="ps3", bufs=2, space="PSUM") as ps3,
        ):
            w1s = cpool.tile([C, H], bf16, tag="w1")
            w2s = cpool.tile([H, H], bf16, tag="w2")
            w3s = cpool.tile([H, H], bf16, tag="w3")
            b1s = cpool.tile([H, 1], f32, tag="b1")
            b2s = cpool.tile([H, 1], f32, tag="b2")
            b3s = cpool.tile([H, 1], f32, tag="b3")
            npads = cpool.tile([H, S], f32, tag="npad")
            wsums = cpool.tile([H, O], f32, tag="wsum")
            wmaxs = cpool.tile([H, O], f32, tag="wmax")
            wmeans = cpool.tile([H, O], f32, tag="wmean")
            bos = cpool.tile([1, O], f32, tag="bo")
            recs = cpool.tile([H, S // H], f32, tag="recip")
            ones = cpool.tile([1, H], f32, tag="ones")

            nc.sync.dma_start(w1s[:], w1)
            nc.sync.dma_start(w2s[:], w2)
            nc.sync.dma_start(w3s[:], w3)
            nc.sync.dma_start(b1s[:], b1)
            nc.sync.dma_start(b2s[:], b2)
            nc.sync.dma_start(b3s[:], b3)
            nc.sync.dma_start(npads[:], npad)
            nc.sync.dma_start(wsums[:], wsum)
            nc.sync.dma_start(wmaxs[:], wmax)
            nc.sync.dma_start(wmeans[:], wmean)
            nc.sync.dma_start(bos[:], bo)
            nc.sync.dma_start(recs[:], recip)
            nc.vector.memset(ones[:], 1.0)

            # Persistent per-slot partials (post-relu sums and maxes) plus
            # the duplicated-pad repair term (-npad_k * h3[:, first_k]).
            sumP = accpool.tile([H, S], f32, tag="sumP")
            maxP = accpool.tile([H, S], f32, tag="maxP")
            padC = accpool.tile([H, S], f32, tag="padC")

            pairs = [tiles[i : i + 2] for i in range(0, len(tiles), 2)]

            pi = 0
            for pair in pairs:
                pcols = sum(d * wt for (_, d, wt, _) in pair)
                pcol0 = pair[0][3]
                xt = xpool.tile([C, PAIR], bf16, tag="xt")
                nc.sync.dma_start(xt[:, :pcols], xT[:, pcol0 : pcol0 + pcols])

                # Layer 1+2: per-tile PSUM, back-to-back matmuls per layer so
                # LDWEIGHTS amortizes and the PE stream stays dense.
                p1s, h1s, p2s, h2s = [], [], [], []
                for k0, d, wt, col0 in pair:
                    tcols = d * wt
                    o0 = col0 - pcol0
                    p1 = ps1.tile([H, MAX_TILE], f32, tag="p1")
                    nc.tensor.matmul(
                        p1[:, :tcols], w1s[:], xt[:, o0 : o0 + tcols]
                    )
                    p1s.append(p1)
                for (k0, d, wt, col0), p1 in zip(pair, p1s):
                    tcols = d * wt
                    h1 = h1pool.tile([H, MAX_TILE], bf16, tag="h1")
                    nc.scalar.activation(
                        h1[:, :tcols], p1[:, :tcols], relu, bias=b1s[:]
                    )
                    h1s.append(h1)
                for (k0, d, wt, col0), h1 in zip(pair, h1s):
                    tcols = d * wt
                    p2 = ps2.tile([H, MAX_TILE], f32, tag="p2")
                    nc.tensor.matmul(p2[:, :tcols], w2s[:], h1[:, :tcols])
                    p2s.append(p2)
                for i, ((k0, d, wt, col0), p2) in enumerate(zip(pair, p2s)):
                    tcols = d * wt
                    h2 = h2pool.tile([H, MAX_TILE], bf16, tag="h2")
                    if (pi + i) % 2 == 0:  # half of relu2 on ScalarE
                        nc.scalar.activation(
                            h2[:, :tcols], p2[:, :tcols], relu, bias=b2s[:]
                        )
                    else:
                        nc.vector.tensor_scalar(
                            h2[:, :tcols], p2[:, :tcols], b2s[:], 0.0,
                            op0=add, op1=amax,
                        )
                    h2s.append(h2)

                # Layer 3 into a 2-bank PSUM pair; one fused relu over both.
                p3 = ps3.tile([H, PAIR], f32, tag="p3")
                offs = []
                for i, ((k0, d, wt, col0), h2) in enumerate(zip(pair, h2s)):
                    tcols = d * wt
                    o3 = i * MAX_TILE  # tile i at its own bank
                    nc.tensor.matmul(
                        p3[:, o3 : o3 + tcols], w3s[:], h2[:, :tcols]
                    )
                    offs.append(o3)
                span = offs[-1] + pair[-1][1] * pair[-1][2]
                h3 = h3pool.tile([H, PAIR], bf16, tag="h3")
                nc.scalar.activation(
                    h3[:, :span], p3[:, :span], relu, bias=b3s[:]
                )

                # Segment reduces: one DVE tree-halving level (2x_1P on bf16
                # halves of each slot) then a half-width 1x reduce per tile.
                tm = scrpool.tile([H, MAX_TILE], bf16, tag="tm")
                ts = scrpool.tile([H, MAX_TILE], bf16, tag="ts")
                for (k0, d, wt, col0), o3 in zip(pair, offs):
                    hw = wt // 2
                    h3v = h3[:, o3 : o3 + d * wt].rearrange(
                        "p (d w) -> p d w", d=d
                    )
                    lo = h3v[:, :, :hw]
                    hi = h3v[:, :, hw:]
                    tmv = tm[:, o3 // 2 : o3 // 2 + d * hw].rearrange(
                        "p (d w) -> p d w", d=d
                    )
                    tsv = ts[:, o3 // 2 : o3 // 2 + d * hw].rearrange(
                        "p (d w) -> p d w", d=d
                    )
                    nc.vector.tensor_tensor(tmv, lo, hi, op=amax)
                    nc.vector.tensor_tensor(tsv, lo, hi, op=add)
                    nc.vector.reduce_max(
                        maxP[:, k0 : k0 + d], tmv, axis=mybir.AxisListType.X
                    )
                    nc.vector.reduce_sum(
                        sumP[:, k0 : k0 + d], tsv, axis=mybir.AxisListType.X
                    )
                    # pad repair term: padC[:,k] = -npad_k * h3[:,first_k]
                    # (npads arrives negated; the epilogue adds padC @ W)
                    h3f = h3v[:, :, 0:1]
                    npv = npads[:, k0 : k0 + d].rearrange(
                        "p (d w) -> p d w", w=1
                    )
                    pcv = padC[:, k0 : k0 + d].rearrange(
                        "p (d w) -> p d w", w=1
                    )
                    nc.vector.tensor_tensor(pcv, h3f, npv, op=amult)
                pi += len(pair)

            # ---- epilogue: out[k, :] = sum_k @ Wsum + max_k @ Wmax
            #                + (sum_k * recip_k) @ Wmean + bo ----
            for ch in range(S // H):  # 2 chunks of 128 segments
                sl = slice(ch * H, (ch + 1) * H)
                pot = ps1.tile([H, MAX_TILE], f32, tag="p1")
                po = pot[:, :O]
                nc.tensor.matmul(po, sumP[:, sl], wsums[:], start=True, stop=False)
                nc.tensor.matmul(po, padC[:, sl], wsums[:], start=False, stop=False)
                nc.tensor.matmul(po, maxP[:, sl], wmaxs[:], start=False, stop=False)
                nc.tensor.matmul(po, ones[:], bos[:], start=False, stop=True)

                pmt = ps2.tile([H, MAX_TILE], f32, tag="p2")
                pm = pmt[:, :O]
                nc.tensor.matmul(pm, sumP[:, sl], wmeans[:], start=True, stop=False)
                nc.tensor.matmul(pm, padC[:, sl], wmeans[:], start=False, stop=True)

                om = h1pool.tile([H, O], f32, tag="om")
                nc.vector.tensor_scalar_mul(om[:], pm, recs[:, ch : ch + 1])
                ot = h2pool.tile([H, O], f32, tag="ot")
                nc.vector.tensor_tensor(ot[:], po, om[:], op=add)
                nc.sync.dma_start(out[sl, :], ot[:])

    nc.compile()
    return nc


def kernel(**inputs):
    x = np.ascontiguousarray(np.asarray(inputs["x"], dtype=np.float32))
    batch = np.asarray(inputs["batch"]).astype(np.int64)

    # ---- fold BN into the linears ----
    W1p, b1p = _fold_bn(
        np.asarray(inputs["W1"]), np.asarray(inputs["b1"]),
        np.asarray(inputs["g1"]), np.asarray(inputs["be1"]),
        np.asarray(inputs["m1"]), np.asarray(inputs["v1"]),
    )
    W2p, b2p = _fold_bn(
        np.asarray(inputs["W2"]), np.asarray(inputs["b2"]),
        np.asarray(inputs["g2"]), np.asarray(inputs["be2"]),
        np.asarray(inputs["m2"]), np.asarray(inputs["v2"]),
    )
    W3p, b3p = _fold_bn(
        np.asarray(inputs["W3"]), np.asarray(inputs["b3"]),
        np.asarray(inputs["g3"]), np.asarray(inputs["be3"]),
        np.asarray(inputs["m3"]), np.asarray(inputs["v3"]),
    )
    Wop, bop = _fold_bn(
        np.asarray(inputs["Wo"]), np.asarray(inputs["bo"]),
        np.asarray(inputs["go"]), np.asarray(inputs["beo"]),
        np.asarray(inputs["mo"]), np.asarray(inputs["vo"]),
    )

    # ---- whole-segment sharding by sorted-width round-robin rank ----
    counts = np.bincount(batch, minlength=NSEG).astype(np.int64)
    assert np.all(batch[:-1] <= batch[1:]), "batch must be sorted"
    order = np.argsort(-counts, kind="stable")  # segment ids, width desc
    slot_w = np.maximum(counts[order[::NCORES][:S]], 1)  # width of rank 8k
    tiles, cols = _plan_tiles(slot_w)

    key = (cols, tuple(slot_w.tolist()))
    if key not in _compiled_cache:
        _compiled_cache[key] = _build_program(tiles, cols)
    nc = _compiled_cache[key]

    # column start / padded width of each slot
    slot_col = np.zeros(S, dtype=np.int64)
    slot_wt = np.zeros(S, dtype=np.int64)
    for k0, d, wt, col0 in tiles:
        for j in range(d):
            slot_col[k0 + j] = col0 + j * wt
        slot_wt[k0 : k0 + d] = wt

    starts = np.searchsorted(batch, np.arange(NSEG), side="left")
    ends = np.searchsorted(batch, np.arange(NSEG), side="right")

    bf = ml_dtypes.bfloat16
    in_maps = []
    for c in range(NCORES):
        segs = order[np.arange(S) * NCORES + c]  # this core's segment ids
        ccnt = counts[segs]
        src = np.full(cols, -1, dtype=np.int64)
        for k in range(S):
            s = segs[k]
            cnt = int(ccnt[k])
            c0 = slot_col[k]
            if cnt:
                src[c0 : c0 + cnt] = np.arange(starts[s], ends[s])
                # pads duplicate the first member (exact for max; sum fixed
                # on device via the npad correction)
                src[c0 + cnt : c0 + slot_wt[k]] = starts[s]
        real = src >= 0
        xTc = np.zeros((C, cols), dtype=bf)
        xTc[:, real] = x[src[real]].T.astype(bf)
        npadc = np.where(ccnt > 0, -(slot_wt - ccnt), 0).astype(np.float32)
        recipc = (1.0 / np.maximum(ccnt, 1.0)).astype(np.float32)
        in_maps.append(
            dict(
                xT=xTc,
                w1=np.ascontiguousarray(W1p.T).astype(bf),
                w2=np.ascontiguousarray(W2p.T).astype(bf),
                w3=np.ascontiguousarray(W3p.T).astype(bf),
                b1=np.ascontiguousarray(b1p[:, None]),
                b2=np.ascontiguousarray(b2p[:, None]),
                b3=np.ascontiguousarray(b3p[:, None]),
                npad=np.ascontiguousarray(
                    np.broadcast_to(npadc[None, :], (H, S))
                ),
                wsum=np.ascontiguousarray(Wop[:, 0:H].T),
                wmax=np.ascontiguousarray(Wop[:, H : 2 * H].T),
                wmean=np.ascontiguousarray(Wop[:, 2 * H : 3 * H].T),
                bo=np.ascontiguousarray(bop[None, :]),
                recip=np.ascontiguousarray(recipc.reshape(S // H, H).T),
            )
        )

    ncores_run = int(os.environ.get("KERNEL_NCORES", str(NCORES)))
    res = bass_utils.run_bass_kernel_spmd(
        nc,
        in_maps[:ncores_run],
        core_ids=list(range(ncores_run)),
        trace=bool(int(os.environ.get("KERNEL_TRACE", "0"))),
        tmpdir=os.environ.get("KERNEL_TRACE_DIR") or None,
    )
    kernel.last_results = res

    out_full = np.zeros((NSEG, O), dtype=np.float32)
    ranks = np.arange(S)
    for c in range(ncores_run):
        out_full[order[ranks * NCORES + c]] = res.results[c]["out"]
    # empty segments: agg == 0, so the output row is just the folded bias
    out_full[counts == 0] = bop
    return out_full


# revision 14
# speedup vs baseline: 1.1961x; 1.1961x over previous
"""DeepSets segment-reduce kernel for 8x Trainium2 NeuronCores.

Strategy (all shapes hardcoded for N=500000, C=H=128, O=64, NSEG=2048):
  - Transposed activation layout: features on SBUF partitions, nodes on the
    free axis, so segment reductions are free-axis operations.
  - Whole-segment sharding: every segment is assigned entirely to one core,
    round-robin by global sorted-width rank.  All 8 cores share an identical
    compile-time slot/tile geometry (SPMD-safe); per-core padding is small.
    No collective is needed - the host gather is the unshard.
  - Encoder BN is folded into the linear weights; each layer is relu(W'x+b').
  - bf16 activations/weights halve DMA bytes and SBUF traffic; PSUM stays
    fp32.  Error budget (2e-2) dwarfs bf16 noise.
  - Pad columns DUPLICATE the slot's first member, so the segment max is
    exact for free; the inflated segment sum is fixed by one rank-1
    subtraction per tile (npad * h3[first]).  Empty segments are patched on
    the host (their output row is the folded bias).
  - Per-slot segment sum AND max are single-instruction DVE tensor_scalar
    ops with accum_out (fold op = op1) reading post-relu bf16 h3 from SBUF.
  - Tiles are processed in pairs: one DMA per pair (2KB/partition lines),
    LDWEIGHTS amortized over both tiles, and relu3 is one ScalarE
    instruction over a 2-bank PSUM pair.
  - Final projection out = [sum|max|mean] @ Wo'.T + bo' runs per core on its
    own 256 segments; mean rides the sum through the Wmean block with a
    per-slot reciprocal row-scale.
"""

import os
import sys

import numpy as np

if "/opt/trn_rl_repo" not in sys.path:
    sys.path.insert(0, "/opt/trn_rl_repo")

import ml_dtypes

import concourse.bacc as bacc
import concourse.mybir as mybir
import concourse.tile as tile
from concourse import bass_utils

EPS = 1e-5
NSEG = 2048
NCORES = 8
C = 128
H = 128
O = 64
S = NSEG // NCORES  # segment slots per core (256)
MAX_TILE = 512  # PSUM bank limit (fp32)

_compiled_cache = {}


def _fold_bn(W, b, g, be, m, v):
    a = g / np.sqrt(v + EPS)
    Wp = W * a[:, None]
    bp = (b - m) * a + be
    return Wp.astype(np.float32), bp.astype(np.float32)


def _plan_tiles(slot_w):
    """Greedy-pack slots (widths descending) into tiles of <=MAX_TILE cols.

    Returns list of (slot_start, n_slots, padded_width, col_start) and the
    total padded column count.
    """
    tiles = []
    col = 0
    k = 0
    n = len(slot_w)
    while k < n:
        # multiples of 4 so bf16 half-slot views stay 4B-aligned (2x_1P)
        wt = (int(slot_w[k]) + 3) & ~3
        assert 0 < wt <= MAX_TILE, f"slot width {wt} unsupported"
        d = min(MAX_TILE // wt, n - k)
        tiles.append((k, d, wt, col))
        col += d * wt
        k += d
    return tiles, col


def _build_program(tiles, cols):
    """Emit the Bass/Tile program shared by all 8 cores."""
    nc = bacc.Bacc(
        "TRN2",
        target_bir_lowering=False,
        debug=False,
        num_devices=NCORES,
    )
    f32 = mybir.dt.float32
    bf16 = mybir.dt.bfloat16

    xT = nc.dram_tensor("xT", [C, cols], bf16, kind="ExternalInput").ap()
    w1 = nc.dram_tensor("w1", [C, H], bf16, kind="ExternalInput").ap()
    w2 = nc.dram_tensor("w2", [H, H], bf16, kind="ExternalInput").ap()
    w3 = nc.dram_tensor("w3", [H, H], bf16, kind="ExternalInput").ap()
    b1 = nc.dram_tensor("b1", [H, 1], f32, kind="ExternalInput").ap()
    b2 = nc.dram_tensor("b2", [H, 1], f32, kind="ExternalInput").ap()
    b3 = nc.dram_tensor("b3", [H, 1], f32, kind="ExternalInput").ap()
    npad = nc.dram_tensor("npad", [H, S], f32, kind="ExternalInput").ap()
    wsum = nc.dram_tensor("wsum", [H, O], f32, kind="ExternalInput").ap()
    wmax = nc.dram_tensor("wmax", [H, O], f32, kind="ExternalInput").ap()
    wmean = nc.dram_tensor("wmean", [H, O], f32, kind="ExternalInput").ap()
    bo = nc.dram_tensor("bo", [1, O], f32, kind="ExternalInput").ap()
    # column ch holds the reciprocals for segment chunk ch (128 slots each)
    recip = nc.dram_tensor("recip", [H, S // H], f32, kind="ExternalInput").ap()
    out = nc.dram_tensor("out", [S, O], f32, kind="ExternalOutput").ap()

    relu = mybir.ActivationFunctionType.Relu
    add = mybir.AluOpType.add
    amax = mybir.AluOpType.max
    asub = mybir.AluOpType.subtract
    amult = mybir.AluOpType.mult

    PAIR = 2 * MAX_TILE

    with tile.TileContext(nc) as tc:
        with (
            tc.tile_pool(name="const", bufs=1) as cpool,
            tc.tile_pool(name="xin", bufs=6) as xpool,
            tc.tile_pool(name="h1", bufs=6) as h1pool,
            tc.tile_pool(name="h2", bufs=6) as h2pool,
            tc.tile_pool(name="h3", bufs=4) as h3pool,
            tc.tile_pool(name="scr", bufs=3) as scrpool,
            tc.tile_pool(name="acc", bufs=1) as accpool,
            tc.tile_pool(name="ps1", bufs=2, space="PSUM") as ps1,
            tc.tile_pool(name="ps2", bufs=2, space="PSUM") as ps2,
            tc.tile_pool(name="ps3", bufs=2, space="PSUM") as ps3,
        ):
            w1s = cpool.tile([C, H], bf16, tag="w1")
            w2s = cpool.tile([H, H], bf16, tag="w2")
            w3s = cpool.tile([H, H], bf16, tag="w3")
            b1s = cpool.tile([H, 1], f32, tag="b1")
            b2s = cpool.tile([H, 1], f32, tag="b2")
            b3s = cpool.tile([H, 1], f32, tag="b3")
            npads = cpool.tile([H, S], f32, tag="npad")
            wsums = cpool.tile([H, O], f32, tag="wsum")
            wmaxs = cpool.tile([H, O], f32, tag="wmax")
            wmeans = cpool.tile([H, O], f32, tag="wmean")
            bos = cpool.tile([1, O], f32, tag="bo")
            recs = cpool.tile([H, S // H], f32, tag="recip")
            ones = cpool.tile([1, H], f32, tag="ones")

            nc.sync.dma_start(w1s[:], w1)
            nc.sync.dma_start(w2s[:], w2)
            nc.sync.dma_start(w3s[:], w3)
            nc.sync.dma_start(b1s[:], b1)
            nc.sync.dma_start(b2s[:], b2)
            nc.sync.dma_start(b3s[:], b3)
            nc.sync.dma_start(npads[:], npad)
            nc.sync.dma_start(wsums[:], wsum)
            nc.sync.dma_start(wmaxs[:], wmax)
            nc.sync.dma_start(wmeans[:], wmean)
            nc.sync.dma_start(bos[:], bo)
            nc.sync.dma_start(recs[:], recip)
            nc.vector.memset(ones[:], 1.0)

            # Persistent per-slot partials (post-relu sums and maxes) plus
            # the duplicated-pad repair term (-npad_k * h3[:, first_k]).
            sumP = accpool.tile([H, S], f32, tag="sumP")
            maxP = accpool.tile([H, S], f32, tag="maxP")
            padC = accpool.tile([H, S], f32, tag="padC")

            pairs = [tiles[i : i + 2] for i in range(0, len(tiles), 2)]

            pi = 0
            for pair in pairs:
                pcols = sum(d * wt for (_, d, wt, _) in pair)
                pcol0 = pair[0][3]
                xt = xpool.tile([C, PAIR], bf16, tag="xt")
                nc.sync.dma_start(xt[:, :pcols], xT[:, pcol0 : pcol0 + pcols])

                # Layer 1+2: per-tile PSUM, back-to-back matmuls per layer so
                # LDWEIGHTS amortizes and the PE stream stays dense.
                p1s, h1s, p2s, h2s = [], [], [], []
                for k0, d, wt, col0 in pair:
                    tcols = d * wt
                    o0 = col0 - pcol0
                    p1 = ps1.tile([H, MAX_TILE], f32, tag="p1")
                    nc.tensor.matmul(
                        p1[:, :tcols], w1s[:], xt[:, o0 : o0 + tcols]
                    )
                    p1s.append(p1)
                for (k0, d, wt, col0), p1 in zip(pair, p1s):
                    tcols = d * wt
                    h1 = h1pool.tile([H, MAX_TILE], bf16, tag="h1")
                    nc.scalar.activation(
                        h1[:, :tcols], p1[:, :tcols], relu, bias=b1s[:]
                    )
                    h1s.append(h1)
                for (k0, d, wt, col0), h1 in zip(pair, h1s):
                    tcols = d * wt
                    p2 = ps2.tile([H, MAX_TILE], f32, tag="p2")
                    nc.tensor.matmul(p2[:, :tcols], w2s[:], h1[:, :tcols])
                    p2s.append(p2)
                for i, ((k0, d, wt, col0), p2) in enumerate(zip(pair, p2s)):
                    tcols = d * wt
                    h2 = h2pool.tile([H, MAX_TILE], bf16, tag="h2")
                    if (pi + i) % 8 < 3:  # 3/8 of relu2 on ScalarE
                        nc.scalar.activation(
                            h2[:, :tcols], p2[:, :tcols], relu, bias=b2s[:]
                        )
                    else:
                        nc.vector.tensor_scalar(
                            h2[:, :tcols], p2[:, :tcols], b2s[:], 0.0,
                            op0=add, op1=amax,
                        )
                    h2s.append(h2)

                # Layer 3 into a 2-bank PSUM pair; one fused relu over both.
                p3 = ps3.tile([H, PAIR], f32, tag="p3")
                offs = []
                for i, ((k0, d, wt, col0), h2) in enumerate(zip(pair, h2s)):
                    tcols = d * wt
                    o3 = i * MAX_TILE  # tile i at its own bank
                    nc.tensor.matmul(
                        p3[:, o3 : o3 + tcols], w3s[:], h2[:, :tcols]
                    )
                    offs.append(o3)
                span = offs[-1] + pair[-1][1] * pair[-1][2]
                h3 = h3pool.tile([H, PAIR], bf16, tag="h3")
                nc.scalar.activation(
                    h3[:, :span], p3[:, :span], relu, bias=b3s[:]
                )

                # Segment reduces: one DVE tree-halving level (2x_1P on bf16
                # halves of each slot) then a half-width 1x reduce per tile.
                tm = scrpool.tile([H, MAX_TILE], bf16, tag="tm")
                ts = scrpool.tile([H, MAX_TILE], bf16, tag="ts")
                for (k0, d, wt, col0), o3 in zip(pair, offs):
                    hw = wt // 2
                    h3v = h3[:, o3 : o3 + d * wt].rearrange(
                        "p (d w) -> p d w", d=d
                    )
                    lo = h3v[:, :, :hw]
                    hi = h3v[:, :, hw:]
                    tmv = tm[:, o3 // 2 : o3 // 2 + d * hw].rearrange(
                        "p (d w) -> p d w", d=d
                    )
                    tsv = ts[:, o3 // 2 : o3 // 2 + d * hw].rearrange(
                        "p (d w) -> p d w", d=d
                    )
                    nc.vector.tensor_tensor(tmv, lo, hi, op=amax)
                    nc.vector.tensor_tensor(tsv, lo, hi, op=add)
                    nc.vector.reduce_max(
                        maxP[:, k0 : k0 + d], tmv, axis=mybir.AxisListType.X
                    )
                    nc.vector.reduce_sum(
                        sumP[:, k0 : k0 + d], tsv, axis=mybir.AxisListType.X
                    )
                    # pad repair term: padC[:,k] = -npad_k * h3[:,first_k]
                    # (npads arrives negated; the epilogue adds padC @ W)
                    h3f = h3v[:, :, 0:1]
                    npv = npads[:, k0 : k0 + d].rearrange(
                        "p (d w) -> p d w", w=1
                    )
                    pcv = padC[:, k0 : k0 + d].rearrange(
                        "p (d w) -> p d w", w=1
                    )
                    nc.vector.tensor_tensor(pcv, h3f, npv, op=amult)
                pi += len(pair)

            # ---- epilogue: out[k, :] = sum_k @ Wsum + max_k @ Wmax
            #                + (sum_k * recip_k) @ Wmean + bo ----
            for ch in range(S // H):  # 2 chunks of 128 segments
                sl = slice(ch * H, (ch + 1) * H)
                pot = ps1.tile([H, MAX_TILE], f32, tag="p1")
                po = pot[:, :O]
                nc.tensor.matmul(po, sumP[:, sl], wsums[:], start=True, stop=False)
                nc.tensor.matmul(po, padC[:, sl], wsums[:], start=False, stop=False)
                nc.tensor.matmul(po, maxP[:, sl], wmaxs[:], start=False, stop=False)
                nc.tensor.matmul(po, ones[:], bos[:], start=False, stop=True)

                pmt = ps2.tile([H, MAX_TILE], f32, tag="p2")
                pm = pmt[:, :O]
                nc.tensor.matmul(pm, sumP[:, sl], wmeans[:], start=True, stop=False)
                nc.tensor.matmul(pm, padC[:, sl], wmeans[:], start=False, stop=True)

                om = h1pool.tile([H, O], f32, tag="om")
                nc.vector.tensor_scalar_mul(om[:], pm, recs[:, ch : ch + 1])
                ot = h2pool.tile([H, O], f32, tag="ot")
                nc.vector.tensor_tensor(ot[:], po, om[:], op=add)
                nc.sync.dma_start(out[sl, :], ot[:])

    nc.compile()
    return nc


def kernel(**inputs):
    x = np.ascontiguousarray(np.asarray(inputs["x"], dtype=np.float32))
    batch = np.asarray(inputs["batch"]).astype(np.int64)

    # ---- fold BN into the linears ----
    W1p, b1p = _fold_bn(
        np.asarray(inputs["W1"]), np.asarray(inputs["b1"]),
        np.asarray(inputs["g1"]), np.asarray(inputs["be1"]),
        np.asarray(inputs["m1"]), np.asarray(inputs["v1"]),
    )
    W2p, b2p = _fold_bn(
        np.asarray(inputs["W2"]), np.asarray(inputs["b2"]),
        np.asarray(inputs["g2"]), np.asarray(inputs["be2"]),
        np.asarray(inputs["m2"]), np.asarray(inputs["v2"]),
    )
    W3p, b3p = _fold_bn(
        np.asarray(inputs["W3"]), np.asarray(inputs["b3"]),
        np.asarray(inputs["g3"]), np.asarray(inputs["be3"]),
        np.asarray(inputs["m3"]), np.asarray(inputs["v3"]),
    )
    Wop, bop = _fold_bn(
        np.asarray(inputs["Wo"]), np.asarray(inputs["bo"]),
        np.asarray(inputs["go"]), np.asarray(inputs["beo"]),
        np.asarray(inputs["mo"]), np.asarray(inputs["vo"]),
    )

    # ---- whole-segment sharding by sorted-width round-robin rank ----
    counts = np.bincount(batch, minlength=NSEG).astype(np.int64)
    assert np.all(batch[:-1] <= batch[1:]), "batch must be sorted"
    order = np.argsort(-counts, kind="stable")  # segment ids, width desc
    slot_w = np.maximum(counts[order[::NCORES][:S]], 1)  # width of rank 8k
    tiles, cols = _plan_tiles(slot_w)

    key = (cols, tuple(slot_w.tolist()))
    if key not in _compiled_cache:
        _compiled_cache[key] = _build_program(tiles, cols)
    nc = _compiled_cache[key]

    # column start / padded width of each slot
    slot_col = np.zeros(S, dtype=np.int64)
    slot_wt = np.zeros(S, dtype=np.int64)
    for k0, d, wt, col0 in tiles:
        for j in range(d):
            slot_col[k0 + j] = col0 + j * wt
        slot_wt[k0 : k0 + d] = wt

    starts = np.searchsorted(batch, np.arange(NSEG), side="left")
    ends = np.searchsorted(batch, np.arange(NSEG), side="right")

    bf = ml_dtypes.bfloat16
    in_maps = []
    for c in range(NCORES):
        segs = order[np.arange(S) * NCORES + c]  # this core's segment ids
        ccnt = counts[segs]
        src = np.full(cols, -1, dtype=np.int64)
        for k in range(S):
            s = segs[k]
            cnt = int(ccnt[k])
            c0 = slot_col[k]
            if cnt:
                src[c0 : c0 + cnt] = np.arange(starts[s], ends[s])
                # pads duplicate the first member (exact for max; sum fixed
                # on device via the npad correction)
                src[c0 + cnt : c0 + slot_wt[k]] = starts[s]
        real = src >= 0
        xTc = np.zeros((C, cols), dtype=bf)
        xTc[:, real] = x[src[real]].T.astype(bf)
        npadc = np.where(ccnt > 0, -(slot_wt - ccnt), 0).astype(np.float32)
        recipc = (1.0 / np.maximum(ccnt, 1.0)).astype(np.float32)
        in_maps.append(
            dict(
                xT=xTc,
                w1=np.ascontiguousarray(W1p.T).astype(bf),
                w2=np.ascontiguousarray(W2p.T).astype(bf),
                w3=np.ascontiguousarray(W3p.T).astype(bf),
                b1=np.ascontiguousarray(b1p[:, None]),
                b2=np.ascontiguousarray(b2p[:, None]),
                b3=np.ascontiguousarray(b3p[:, None]),
                npad=np.ascontiguousarray(
                    np.broadcast_to(npadc[None, :], (H, S))
                ),
                wsum=np.ascontiguousarray(Wop[:, 0:H].T),
                wmax=np.ascontiguousarray(Wop[:, H : 2 * H].T),
                wmean=np.ascontiguousarray(Wop[:, 2 * H : 3 * H].T),
                bo=np.ascontiguousarray(bop[None, :]),
                recip=np.ascontiguousarray(recipc.reshape(S // H, H).T),
            )
        )

    ncores_run = int(os.environ.get("KERNEL_NCORES", str(NCORES)))
    res = bass_utils.run_bass_kernel_spmd(
        nc,
        in_maps[:ncores_run],
        core_ids=list(range(ncores_run)),
        trace=bool(int(os.environ.get("KERNEL_TRACE", "0"))),
        tmpdir=os.environ.get("KERNEL_TRACE_DIR") or None,
    )
    kernel.last_results = res

    out_full = np.zeros((NSEG, O), dtype=np.float32)
    ranks = np.arange(S)
    for c in range(ncores_run):
        out_full[order[ranks * NCORES + c]] = res.results[c]["out"]
    # empty segments: agg == 0, so the output row is just the folded bias
    out_full[counts == 0] = bop
    return out_full


# revision 20
# speedup vs baseline: 1.2044x; 1.0069x over previous
"""DeepSets segment-reduce kernel for 8x Trainium2 NeuronCores.

Strategy (all shapes hardcoded for N=500000, C=H=128, O=64, NSEG=2048):
  - Transposed activation layout: features on SBUF partitions, nodes on the
    free axis, so segment reductions are free-axis operations.
  - Whole-segment sharding: every segment is assigned entirely to one core,
    round-robin by global sorted-width rank.  All 8 cores share an identical
    compile-time slot/tile geometry (SPMD-safe); per-core padding is small.
    No collective is needed - the host gather is the unshard.
  - Encoder BN is folded into the linear weights; each layer is relu(W'x+b').
  - bf16 activations/weights halve DMA bytes and SBUF traffic; PSUM stays
    fp32.  Error budget (2e-2) dwarfs bf16 noise.
  - Pad columns DUPLICATE the slot's first member, so the segment max is
    exact for free; the inflated segment sum is fixed by one rank-1
    subtraction per tile (npad * h3[first]).  Empty segments are patched on
    the host (their output row is the folded bias).
  - Per-slot segment sum AND max are single-instruction DVE tensor_scalar
    ops with accum_out (fold op = op1) reading post-relu bf16 h3 from SBUF.
  - Tiles are processed in pairs: one DMA per pair (2KB/partition lines),
    LDWEIGHTS amortized over both tiles, and relu3 is one ScalarE
    instruction over a 2-bank PSUM pair.
  - Final projection out = [sum|max|mean] @ Wo'.T + bo' runs per core on its
    own 256 segments; mean rides the sum through the Wmean block with a
    per-slot reciprocal row-scale.
"""

import os
import sys

import numpy as np

if "/opt/trn_rl_repo" not in sys.path:
    sys.path.insert(0, "/opt/trn_rl_repo")

import ml_dtypes

import concourse.bacc as bacc
import concourse.mybir as mybir
import concourse.tile as tile
from concourse import bass_utils

EPS = 1e-5
NSEG = 2048
NCORES = 8
C = 128
H = 128
O = 64
S = NSEG // NCORES  # segment slots per core (256)
MAX_TILE = 512  # PSUM bank limit (fp32)

_compiled_cache = {}


def _fold_bn(W, b, g, be, m, v):
    a = g / np.sqrt(v + EPS)
    Wp = W * a[:, None]
    bp = (b - m) * a + be
    return Wp.astype(np.float32), bp.astype(np.float32)


def _plan_tiles(slot_w):
    """Greedy-pack slots (widths descending) into tiles of <=MAX_TILE cols.

    Returns list of (slot_start, n_slots, padded_width, col_start) and the
    total padded column count.
    """
    tiles = []
    col = 0
    k = 0
    n = len(slot_w)
    while k < n:
        # multiples of 4 so bf16 half-slot views stay 4B-aligned (2x_1P)
        wt = (int(slot_w[k]) + 3) & ~3
        assert 0 < wt <= MAX_TILE, f"slot width {wt} unsupported"
        d = min(MAX_TILE // wt, n - k)
        tiles.append((k, d, wt, col))
        col += d * wt
        k += d
    return tiles, col


def _build_program(tiles, cols):
    """Emit the Bass/Tile program shared by all 8 cores."""
    nc = bacc.Bacc(
        "TRN2",
        target_bir_lowering=False,
        debug=False,
        num_devices=NCORES,
    )
    f32 = mybir.dt.float32
    bf16 = mybir.dt.bfloat16

    xT = nc.dram_tensor("xT", [C, cols], bf16, kind="ExternalInput").ap()
    w1 = nc.dram_tensor("w1", [C, H], bf16, kind="ExternalInput").ap()
    w2 = nc.dram_tensor("w2", [H, H], bf16, kind="ExternalInput").ap()
    w3 = nc.dram_tensor("w3", [H, H], bf16, kind="ExternalInput").ap()
    b1 = nc.dram_tensor("b1", [H, 1], f32, kind="ExternalInput").ap()
    b2 = nc.dram_tensor("b2", [H, 1], f32, kind="ExternalInput").ap()
    b3 = nc.dram_tensor("b3", [H, 1], f32, kind="ExternalInput").ap()
    npad = nc.dram_tensor("npad", [H, S], f32, kind="ExternalInput").ap()
    wsum = nc.dram_tensor("wsum", [H, O], f32, kind="ExternalInput").ap()
    wmax = nc.dram_tensor("wmax", [H, O], f32, kind="ExternalInput").ap()
    wmean = nc.dram_tensor("wmean", [H, O], f32, kind="ExternalInput").ap()
    bo = nc.dram_tensor("bo", [1, O], f32, kind="ExternalInput").ap()
    # column ch holds the reciprocals for segment chunk ch (128 slots each)
    recip = nc.dram_tensor("recip", [H, S // H], f32, kind="ExternalInput").ap()
    out = nc.dram_tensor("out", [S, O], f32, kind="ExternalOutput").ap()

    relu = mybir.ActivationFunctionType.Relu
    add = mybir.AluOpType.add
    amax = mybir.AluOpType.max
    asub = mybir.AluOpType.subtract
    amult = mybir.AluOpType.mult

    PAIR = 2 * MAX_TILE

    with tile.TileContext(nc) as tc:
        with (
            tc.tile_pool(name="const", bufs=1) as cpool,
            tc.tile_pool(name="xin", bufs=6) as xpool,
            tc.tile_pool(name="h1", bufs=6) as h1pool,
            tc.tile_pool(name="h2", bufs=6) as h2pool,
            tc.tile_pool(name="h3", bufs=4) as h3pool,
            tc.tile_pool(name="scr", bufs=3) as scrpool,
            tc.tile_pool(name="acc", bufs=1) as accpool,
            tc.tile_pool(name="ps1", bufs=2, space="PSUM") as ps1,
            tc.tile_pool(name="ps2", bufs=2, space="PSUM") as ps2,
            tc.tile_pool(name="ps3", bufs=2, space="PSUM") as ps3,
        ):
            w1s = cpool.tile([C, H], bf16, tag="w1")
            w2s = cpool.tile([H, H], bf16, tag="w2")
            w3s = cpool.tile([H, H], bf16, tag="w3")
            b1s = cpool.tile([H, 1], f32, tag="b1")
            b2s = cpool.tile([H, 1], f32, tag="b2")
            b3s = cpool.tile([H, 1], f32, tag="b3")
            npads = cpool.tile([H, S], f32, tag="npad")
            wsums = cpool.tile([H, O], f32, tag="wsum")
            wmaxs = cpool.tile([H, O], f32, tag="wmax")
            wmeans = cpool.tile([H, O], f32, tag="wmean")
            bos = cpool.tile([1, O], f32, tag="bo")
            recs = cpool.tile([H, S // H], f32, tag="recip")
            ones = cpool.tile([1, H], f32, tag="ones")

            nc.sync.dma_start(w1s[:], w1)
            nc.sync.dma_start(w2s[:], w2)
            nc.sync.dma_start(w3s[:], w3)
            nc.sync.dma_start(b1s[:], b1)
            nc.sync.dma_start(b2s[:], b2)
            nc.sync.dma_start(b3s[:], b3)
            nc.sync.dma_start(npads[:], npad)
            nc.sync.dma_start(wsums[:], wsum)
            nc.sync.dma_start(wmaxs[:], wmax)
            nc.sync.dma_start(wmeans[:], wmean)
            nc.sync.dma_start(bos[:], bo)
            nc.sync.dma_start(recs[:], recip)
            nc.vector.memset(ones[:], 1.0)

            # Persistent per-slot partials (post-relu sums and maxes) plus
            # the duplicated-pad repair term (-npad_k * h3[:, first_k]).
            sumP = accpool.tile([H, S], f32, tag="sumP")
            maxP = accpool.tile([H, S], f32, tag="maxP")
            padC = accpool.tile([H, S], f32, tag="padC")

            pairs = [tiles[i : i + 2] for i in range(0, len(tiles), 2)]

            pi = 0
            for pair in pairs:
                pcols = sum(d * wt for (_, d, wt, _) in pair)
                pcol0 = pair[0][3]
                xt = xpool.tile([C, PAIR], bf16, tag="xt")
                nc.sync.dma_start(xt[:, :pcols], xT[:, pcol0 : pcol0 + pcols])

                # Layer 1+2: per-tile PSUM, back-to-back matmuls per layer so
                # LDWEIGHTS amortizes and the PE stream stays dense.
                p1s, h1s, p2s, h2s = [], [], [], []
                for k0, d, wt, col0 in pair:
                    tcols = d * wt
                    o0 = col0 - pcol0
                    p1 = ps1.tile([H, MAX_TILE], f32, tag="p1")
                    nc.tensor.matmul(
                        p1[:, :tcols], w1s[:], xt[:, o0 : o0 + tcols]
                    )
                    p1s.append(p1)
                for (k0, d, wt, col0), p1 in zip(pair, p1s):
                    tcols = d * wt
                    h1 = h1pool.tile([H, MAX_TILE], bf16, tag="h1")
                    nc.scalar.activation(
                        h1[:, :tcols], p1[:, :tcols], relu, bias=b1s[:]
                    )
                    h1s.append(h1)
                for (k0, d, wt, col0), h1 in zip(pair, h1s):
                    tcols = d * wt
                    p2 = ps2.tile([H, MAX_TILE], f32, tag="p2")
                    nc.tensor.matmul(p2[:, :tcols], w2s[:], h1[:, :tcols])
                    p2s.append(p2)
                for i, ((k0, d, wt, col0), p2) in enumerate(zip(pair, p2s)):
                    tcols = d * wt
                    h2 = h2pool.tile([H, MAX_TILE], bf16, tag="h2")
                    if (pi + i) % 8 < 3:  # 3/8 of relu2 on ScalarE
                        nc.scalar.activation(
                            h2[:, :tcols], p2[:, :tcols], relu, bias=b2s[:]
                        )
                    else:
                        nc.vector.tensor_scalar(
                            h2[:, :tcols], p2[:, :tcols], b2s[:], 0.0,
                            op0=add, op1=amax,
                        )
                    h2s.append(h2)

                # Layer 3 into a 2-bank PSUM pair; one fused relu over both.
                p3 = ps3.tile([H, PAIR], f32, tag="p3")
                offs = []
                for i, ((k0, d, wt, col0), h2) in enumerate(zip(pair, h2s)):
                    tcols = d * wt
                    o3 = i * MAX_TILE  # tile i at its own bank
                    nc.tensor.matmul(
                        p3[:, o3 : o3 + tcols], w3s[:], h2[:, :tcols]
                    )
                    offs.append(o3)
                span = offs[-1] + pair[-1][1] * pair[-1][2]
                h3 = h3pool.tile([H, PAIR], bf16, tag="h3")
                nc.scalar.activation(
                    h3[:, :span], p3[:, :span], relu, bias=b3s[:]
                )

                # Segment reduces: one DVE tree-halving level (2x_1P on bf16
                # halves of each slot) then a half-width 1x reduce per tile.
                tm = scrpool.tile([H, MAX_TILE], bf16, tag="tm")
                ts = scrpool.tile([H, MAX_TILE], bf16, tag="ts")
                for (k0, d, wt, col0), o3 in zip(pair, offs):
                    hw = wt // 2
                    h3v = h3[:, o3 : o3 + d * wt].rearrange(
                        "p (d w) -> p d w", d=d
                    )
                    lo = h3v[:, :, :hw]
                    hi = h3v[:, :, hw:]
                    tmv = tm[:, o3 // 2 : o3 // 2 + d * hw].rearrange(
                        "p (d w) -> p d w", d=d
                    )
                    tsv = ts[:, o3 // 2 : o3 // 2 + d * hw].rearrange(
                        "p (d w) -> p d w", d=d
                    )
                    nc.vector.tensor_tensor(tmv, lo, hi, op=amax)
                    nc.vector.tensor_tensor(tsv, lo, hi, op=add)
                    nc.vector.reduce_max(
                        maxP[:, k0 : k0 + d], tmv, axis=mybir.AxisListType.X
                    )
                    nc.vector.reduce_sum(
                        sumP[:, k0 : k0 + d], tsv, axis=mybir.AxisListType.X
                    )
                    # pad repair term: padC[:,k] = -npad_k * h3[:,first_k]
                    # (npads arrives negated; the epilogue adds padC @ W)
                    h3f = h3v[:, :, 0:1]
                    npv = npads[:, k0 : k0 + d].rearrange(
                        "p (d w) -> p d w", w=1
                    )
                    pcv = padC[:, k0 : k0 + d].rearrange(
                        "p (d w) -> p d w", w=1
                    )
                    nc.vector.tensor_tensor(pcv, h3f, npv, op=amult)
                pi += len(pair)

            # ---- epilogue: out[k, :] = sum_k @ Wsum + max_k @ Wmax
            #                + (sum_k * recip_k) @ Wmean + bo ----
            for ch in range(S // H):  # 2 chunks of 128 segments
                sl = slice(ch * H, (ch + 1) * H)
                pot = ps1.tile([H, MAX_TILE], f32, tag="p1")
                po = pot[:, :O]
                nc.tensor.matmul(po, sumP[:, sl], wsums[:], start=True, stop=False)
                nc.tensor.matmul(po, padC[:, sl], wsums[:], start=False, stop=False)
                nc.tensor.matmul(po, maxP[:, sl], wmaxs[:], start=False, stop=False)
                nc.tensor.matmul(po, ones[:], bos[:], start=False, stop=True)

                pmt = ps2.tile([H, MAX_TILE], f32, tag="p2")
                pm = pmt[:, :O]
                nc.tensor.matmul(pm, sumP[:, sl], wmeans[:], start=True, stop=False)
                nc.tensor.matmul(pm, padC[:, sl], wmeans[:], start=False, stop=True)

                om = h1pool.tile([H, O], f32, tag="om")
                nc.vector.tensor_scalar_mul(om[:], pm, recs[:, ch : ch + 1])
                ot = h2pool.tile([H, O], f32, tag="ot")
                nc.vector.tensor_tensor(ot[:], po, om[:], op=add)
                nc.sync.dma_start(out[sl, :], ot[:])

    nc.compile()
    return nc


def kernel(**inputs):
    x = np.ascontiguousarray(np.asarray(inputs["x"], dtype=np.float32))
    batch = np.asarray(inputs["batch"]).astype(np.int64)

    # ---- fold BN into the linears ----
    W1p, b1p = _fold_bn(
        np.asarray(inputs["W1"]), np.asarray(inputs["b1"]),
        np.asarray(inputs["g1"]), np.asarray(inputs["be1"]),
        np.asarray(inputs["m1"]), np.asarray(inputs["v1"]),
    )
    W2p, b2p = _fold_bn(
        np.asarray(inputs["W2"]), np.asarray(inputs["b2"]),
        np.asarray(inputs["g2"]), np.asarray(inputs["be2"]),
        np.asarray(inputs["m2"]), np.asarray(inputs["v2"]),
    )
    W3p, b3p = _fold_bn(
        np.asarray(inputs["W3"]), np.asarray(inputs["b3"]),
        np.asarray(inputs["g3"]), np.asarray(inputs["be3"]),
        np.asarray(inputs["m3"]), np.asarray(inputs["v3"]),
    )
    Wop, bop = _fold_bn(
        np.asarray(inputs["Wo"]), np.asarray(inputs["bo"]),
        np.asarray(inputs["go"]), np.asarray(inputs["beo"]),
        np.asarray(inputs["mo"]), np.asarray(inputs["vo"]),
    )

    # ---- whole-segment sharding by sorted-width round-robin rank ----
    counts = np.bincount(batch, minlength=NSEG).astype(np.int64)
    assert np.all(batch[:-1] <= batch[1:]), "batch must be sorted"
    order = np.argsort(-counts, kind="stable")  # segment ids, width desc
    slot_w = np.maximum(counts[order[::NCORES][:S]], 1)  # width of rank 8k
    tiles, cols = _plan_tiles(slot_w)

    key = (cols, tuple(slot_w.tolist()))
    if key not in _compiled_cache:
        _compiled_cache[key] = _build_program(tiles, cols)
    nc = _compiled_cache[key]

    # column start / padded width of each slot
    slot_col = np.zeros(S, dtype=np.int64)
    slot_wt = np.zeros(S, dtype=np.int64)
    for k0, d, wt, col0 in tiles:
        for j in range(d):
            slot_col[k0 + j] = col0 + j * wt
        slot_wt[k0 : k0 + d] = wt

    starts = np.searchsorted(batch, np.arange(NSEG), side="left")
    ends = np.searchsorted(batch, np.arange(NSEG), side="right")

    bf = ml_dtypes.bfloat16
    in_maps = []
    for c in range(NCORES):
        segs = order[np.arange(S) * NCORES + c]  # this core's segment ids
        ccnt = counts[segs]
        src = np.full(cols, -1, dtype=np.int64)
        for k in range(S):
            s = segs[k]
            cnt = int(ccnt[k])
            c0 = slot_col[k]
            if cnt:
                src[c0 : c0 + cnt] = np.arange(starts[s], ends[s])
                # pads duplicate the first member (exact for max; sum fixed
                # on device via the npad correction)
                src[c0 + cnt : c0 + slot_wt[k]] = starts[s]
        real = src >= 0
        xTc = np.zeros((C, cols), dtype=bf)
        xTc[:, real] = x[src[real]].T.astype(bf)
        npadc = np.where(ccnt > 0, -(slot_wt - ccnt), 0).astype(np.float32)
        recipc = (1.0 / np.maximum(ccnt, 1.0)).astype(np.float32)
        in_maps.append(
            dict(
                xT=xTc,
                w1=np.ascontiguousarray(W1p.T).astype(bf),
                w2=np.ascontiguousarray(W2p.T).astype(bf),
                w3=np.ascontiguousarray(W3p.T).astype(bf),
                b1=np.ascontiguousarray(b1p[:, None]),
                b2=np.ascontiguousarray(b2p[:, None]),
                b3=np.ascontiguousarray(b3p[:, None]),
                npad=np.ascontiguousarray(
                    np.broadcast_to(npadc[None, :], (H, S))
                ),
                wsum=np.ascontiguousarray(Wop[:, 0:H].T),
                wmax=np.ascontiguousarray(Wop[:, H : 2 * H].T),
                wmean=np.ascontiguousarray(Wop[:, 2 * H : 3 * H].T),
                bo=np.ascontiguousarray(bop[None, :]),
                recip=np.ascontiguousarray(recipc.reshape(S // H, H).T),
            )
        )

    ncores_run = int(os.environ.get("KERNEL_NCORES", str(NCORES)))
    res = bass_utils.run_bass_kernel_spmd(
        nc,
        in_maps[:ncores_run],
        core_ids=list(range(ncores_run)),
        trace=bool(int(os.environ.get("KERNEL_TRACE", "0"))),
        tmpdir=os.environ.get("KERNEL_TRACE_DIR") or None,
    )
    kernel.last_results = res

    out_full = np.zeros((NSEG, O), dtype=np.float32)
    ranks = np.arange(S)
    for c in range(ncores_run):
        out_full[order[ranks * NCORES + c]] = res.results[c]["out"]
    # empty segments: agg == 0, so the output row is just the folded bias
    out_full[counts == 0] = bop
    return out_full


# revision 24
# speedup vs baseline: 1.3694x; 1.1370x over previous
"""DeepSets segment-reduce kernel for 8x Trainium2 NeuronCores.

Strategy (all shapes hardcoded for N=500000, C=H=128, O=64, NSEG=2048):
  - Transposed activation layout: features on SBUF partitions, nodes on the
    free axis, so segment reductions are free-axis operations.
  - Whole-segment sharding: every segment is assigned entirely to one core,
    round-robin by global sorted-width rank.  All 8 cores share an identical
    compile-time slot/tile geometry (SPMD-safe); per-core padding is small.
    No collective is needed - the host gather is the unshard.
  - Encoder BN is folded into the linear weights; each layer is relu(W'x+b').
  - bf16 activations/weights halve DMA bytes and SBUF traffic; PSUM stays
    fp32.  Error budget (2e-2) dwarfs bf16 noise.
  - Pad columns DUPLICATE the slot's first member, so the segment max is
    exact for free; the inflated segment sum is fixed by one rank-1
    subtraction per tile (npad * h3[first]).  Empty segments are patched on
    the host (their output row is the folded bias).
  - Per-slot segment sum AND max are single-instruction DVE tensor_scalar
    ops with accum_out (fold op = op1) reading post-relu bf16 h3 from SBUF.
  - Tiles are processed in pairs: one DMA per pair (2KB/partition lines),
    LDWEIGHTS amortized over both tiles, and relu3 is one ScalarE
    instruction over a 2-bank PSUM pair.
  - Final projection out = [sum|max|mean] @ Wo'.T + bo' runs per core on its
    own 256 segments; mean rides the sum through the Wmean block with a
    per-slot reciprocal row-scale.
"""

import os
import sys

import numpy as np

if "/opt/trn_rl_repo" not in sys.path:
    sys.path.insert(0, "/opt/trn_rl_repo")

import ml_dtypes

import concourse.bacc as bacc
import concourse.mybir as mybir
import concourse.tile as tile
from concourse import bass_utils

EPS = 1e-5
NSEG = 2048
NCORES = 8
C = 128
H = 128
O = 64
S = NSEG // NCORES  # segment slots per core (256)
MAX_TILE = 512  # PSUM bank limit (fp32)

_compiled_cache = {}


def _fold_bn(W, b, g, be, m, v):
    a = g / np.sqrt(v + EPS)
    Wp = W * a[:, None]
    bp = (b - m) * a + be
    return Wp.astype(np.float32), bp.astype(np.float32)


def _plan_tiles(slot_w):
    """Greedy-pack slots (widths descending) into tiles of <=MAX_TILE cols.

    Returns list of (slot_start, n_slots, padded_width, col_start) and the
    total padded column count.
    """
    tiles = []
    col = 0
    k = 0
    n = len(slot_w)
    while k < n:
        # multiples of 4 so bf16 half-slot views stay 4B-aligned (2x_1P)
        wt = (int(slot_w[k]) + 3) & ~3
        assert 0 < wt <= MAX_TILE, f"slot width {wt} unsupported"
        d = min(MAX_TILE // wt, n - k)
        tiles.append((k, d, wt, col))
        col += d * wt
        k += d
    return tiles, col


def _build_program(tiles, cols):
    """Emit the Bass/Tile program shared by all 8 cores."""
    nc = bacc.Bacc(
        "TRN2",
        target_bir_lowering=False,
        debug=False,
        num_devices=NCORES,
    )
    f32 = mybir.dt.float32
    bf16 = mybir.dt.bfloat16

    xT = nc.dram_tensor("xT", [C, cols], bf16, kind="ExternalInput").ap()
    w1 = nc.dram_tensor("w1", [C, H], bf16, kind="ExternalInput").ap()
    w2 = nc.dram_tensor("w2", [H, H], bf16, kind="ExternalInput").ap()
    w3 = nc.dram_tensor("w3", [H, H], bf16, kind="ExternalInput").ap()
    b1 = nc.dram_tensor("b1", [H, 1], f32, kind="ExternalInput").ap()
    b2 = nc.dram_tensor("b2", [H, 1], f32, kind="ExternalInput").ap()
    b3 = nc.dram_tensor("b3", [H, 1], f32, kind="ExternalInput").ap()
    npad = nc.dram_tensor("npad", [H, S], f32, kind="ExternalInput").ap()
    wsum = nc.dram_tensor("wsum", [H, O], f32, kind="ExternalInput").ap()
    wmax = nc.dram_tensor("wmax", [H, O], f32, kind="ExternalInput").ap()
    wmean = nc.dram_tensor("wmean", [H, O], f32, kind="ExternalInput").ap()
    bo = nc.dram_tensor("bo", [1, O], f32, kind="ExternalInput").ap()
    # column ch holds the reciprocals for segment chunk ch (128 slots each)
    recip = nc.dram_tensor("recip", [H, S // H], f32, kind="ExternalInput").ap()
    out = nc.dram_tensor("out", [S, O], f32, kind="ExternalOutput").ap()

    relu = mybir.ActivationFunctionType.Relu
    add = mybir.AluOpType.add
    amax = mybir.AluOpType.max
    asub = mybir.AluOpType.subtract
    amult = mybir.AluOpType.mult

    PAIR = 2 * MAX_TILE

    with tile.TileContext(nc) as tc:
        with (
            tc.tile_pool(name="const", bufs=1) as cpool,
            tc.tile_pool(name="xin", bufs=8) as xpool,
            tc.tile_pool(name="h1", bufs=6) as h1pool,
            tc.tile_pool(name="h2", bufs=8) as h2pool,
            tc.tile_pool(name="h3", bufs=6) as h3pool,
            tc.tile_pool(name="scr", bufs=4) as scrpool,
            tc.tile_pool(name="acc", bufs=1) as accpool,
            tc.tile_pool(name="ps1", bufs=2, space="PSUM") as ps1,
            tc.tile_pool(name="ps2", bufs=2, space="PSUM") as ps2,
            tc.tile_pool(name="ps3", bufs=1, space="PSUM") as ps3,
        ):
            w1s = cpool.tile([C, H], bf16, tag="w1")
            w2s = cpool.tile([H, H], bf16, tag="w2")
            w3s = cpool.tile([H, H], bf16, tag="w3")
            b1s = cpool.tile([H, 1], f32, tag="b1")
            b2s = cpool.tile([H, 1], f32, tag="b2")
            b3s = cpool.tile([H, 1], f32, tag="b3")
            npads = cpool.tile([H, S], f32, tag="npad")
            wsums = cpool.tile([H, O], f32, tag="wsum")
            wmaxs = cpool.tile([H, O], f32, tag="wmax")
            wmeans = cpool.tile([H, O], f32, tag="wmean")
            bos = cpool.tile([1, O], f32, tag="bo")
            recs = cpool.tile([H, S // H], f32, tag="recip")
            ones = cpool.tile([1, H], f32, tag="ones")

            nc.sync.dma_start(w1s[:], w1)
            nc.sync.dma_start(w2s[:], w2)
            nc.sync.dma_start(w3s[:], w3)
            nc.sync.dma_start(b1s[:], b1)
            nc.sync.dma_start(b2s[:], b2)
            nc.sync.dma_start(b3s[:], b3)
            nc.sync.dma_start(npads[:], npad)
            nc.sync.dma_start(wsums[:], wsum)
            nc.sync.dma_start(wmaxs[:], wmax)
            nc.sync.dma_start(wmeans[:], wmean)
            nc.sync.dma_start(bos[:], bo)
            nc.sync.dma_start(recs[:], recip)
            nc.vector.memset(ones[:], 1.0)

            # Persistent per-slot partials (post-relu sums and maxes) plus
            # the duplicated-pad repair term (-npad_k * h3[:, first_k]).
            sumP = accpool.tile([H, S], f32, tag="sumP")
            maxP = accpool.tile([H, S], f32, tag="maxP")
            padC = accpool.tile([H, S], f32, tag="padC")

            pairs = [tiles[i : i + 2] for i in range(0, len(tiles), 2)]

            pi = 0
            for pair in pairs:
                pcols = sum(d * wt for (_, d, wt, _) in pair)
                pcol0 = pair[0][3]
                xt = xpool.tile([C, PAIR], bf16, tag="xt")
                nc.sync.dma_start(xt[:, :pcols], xT[:, pcol0 : pcol0 + pcols])

                # Layer 1 into a 2-bank pair; one fused relu over both tiles.
                p1 = ps1.tile([H, PAIR], f32, tag="p1")
                for i, (k0, d, wt, col0) in enumerate(pair):
                    tcols = d * wt
                    o0 = col0 - pcol0
                    nc.tensor.matmul(
                        p1[:, i * MAX_TILE : i * MAX_TILE + tcols],
                        w1s[:], xt[:, o0 : o0 + tcols],
                    )
                span1 = (len(pair) - 1) * MAX_TILE + pair[-1][1] * pair[-1][2]
                h1 = h1pool.tile([H, PAIR], bf16, tag="h1")
                nc.scalar.activation(
                    h1[:, :span1], p1[:, :span1], relu, bias=b1s[:]
                )

                p2s, h2s = [], []
                for i, (k0, d, wt, col0) in enumerate(pair):
                    tcols = d * wt
                    p2 = ps2.tile([H, MAX_TILE], f32, tag="p2")
                    nc.tensor.matmul(
                        p2[:, :tcols], w2s[:],
                        h1[:, i * MAX_TILE : i * MAX_TILE + tcols],
                    )
                    p2s.append(p2)
                for i, ((k0, d, wt, col0), p2) in enumerate(zip(pair, p2s)):
                    tcols = d * wt
                    h2 = h2pool.tile([H, MAX_TILE], bf16, tag="h2")
                    if (pi + i) % 16 < 9:  # 9/16 of relu2 on ScalarE
                        nc.scalar.activation(
                            h2[:, :tcols], p2[:, :tcols], relu, bias=b2s[:]
                        )
                    else:
                        nc.vector.tensor_scalar(
                            h2[:, :tcols], p2[:, :tcols], b2s[:], 0.0,
                            op0=add, op1=amax,
                        )
                    h2s.append(h2)

                # Layer 3 into a 2-bank PSUM pair; one fused relu over both.
                p3 = ps3.tile([H, PAIR], f32, tag="p3")
                offs = []
                for i, ((k0, d, wt, col0), h2) in enumerate(zip(pair, h2s)):
                    tcols = d * wt
                    o3 = i * MAX_TILE  # tile i at its own bank
                    nc.tensor.matmul(
                        p3[:, o3 : o3 + tcols], w3s[:], h2[:, :tcols]
                    )
                    offs.append(o3)
                span = offs[-1] + pair[-1][1] * pair[-1][2]
                h3 = h3pool.tile([H, PAIR], bf16, tag="h3")
                nc.scalar.activation(
                    h3[:, :span], p3[:, :span], relu, bias=b3s[:]
                )

                # Segment reduces: one DVE tree-halving level (2x_1P on bf16
                # halves of each slot) then a half-width 1x reduce per tile.
                tm = scrpool.tile([H, MAX_TILE], bf16, tag="tm")
                ts = scrpool.tile([H, MAX_TILE], bf16, tag="ts")
                for (k0, d, wt, col0), o3 in zip(pair, offs):
                    hw = wt // 2
                    h3v = h3[:, o3 : o3 + d * wt].rearrange(
                        "p (d w) -> p d w", d=d
                    )
                    lo = h3v[:, :, :hw]
                    hi = h3v[:, :, hw:]
                    tmv = tm[:, o3 // 2 : o3 // 2 + d * hw].rearrange(
                        "p (d w) -> p d w", d=d
                    )
                    tsv = ts[:, o3 // 2 : o3 // 2 + d * hw].rearrange(
                        "p (d w) -> p d w", d=d
                    )
                    nc.vector.tensor_tensor(tmv, lo, hi, op=amax)
                    nc.vector.tensor_tensor(tsv, lo, hi, op=add)
                    nc.vector.reduce_max(
                        maxP[:, k0 : k0 + d], tmv, axis=mybir.AxisListType.X
                    )
                    nc.vector.reduce_sum(
                        sumP[:, k0 : k0 + d], tsv, axis=mybir.AxisListType.X
                    )
                    # pad repair term: padC[:,k] = -npad_k * h3[:,first_k]
                    # (npads arrives negated; the epilogue adds padC @ W)
                    h3f = h3v[:, :, 0:1]
                    npv = npads[:, k0 : k0 + d].rearrange(
                        "p (d w) -> p d w", w=1
                    )
                    pcv = padC[:, k0 : k0 + d].rearrange(
                        "p (d w) -> p d w", w=1
                    )
                    nc.vector.tensor_tensor(pcv, h3f, npv, op=amult)
                pi += len(pair)

            # ---- epilogue: out[k, :] = sum_k @ Wsum + max_k @ Wmax
            #                + (sum_k * recip_k) @ Wmean + bo ----
            for ch in range(S // H):  # 2 chunks of 128 segments
                sl = slice(ch * H, (ch + 1) * H)
                pot = ps1.tile([H, PAIR], f32, tag="p1")
                po = pot[:, :O]
                nc.tensor.matmul(po, sumP[:, sl], wsums[:], start=True, stop=False)
                nc.tensor.matmul(po, padC[:, sl], wsums[:], start=False, stop=False)
                nc.tensor.matmul(po, maxP[:, sl], wmaxs[:], start=False, stop=False)
                nc.tensor.matmul(po, ones[:], bos[:], start=False, stop=True)

                pmt = ps2.tile([H, MAX_TILE], f32, tag="p2")
                pm = pmt[:, :O]
                nc.tensor.matmul(pm, sumP[:, sl], wmeans[:], start=True, stop=False)
                nc.tensor.matmul(pm, padC[:, sl], wmeans[:], start=False, stop=True)

                om = h1pool.tile([H, O], f32, tag="om")
                nc.vector.tensor_scalar_mul(om[:], pm, recs[:, ch : ch + 1])
                ot = h2pool.tile([H, O], f32, tag="ot")
                nc.vector.tensor_tensor(ot[:], po, om[:], op=add)
                nc.sync.dma_start(out[sl, :], ot[:])

    nc.compile()
    return nc


def kernel(**inputs):
    x = np.ascontiguousarray(np.asarray(inputs["x"], dtype=np.float32))
    batch = np.asarray(inputs["batch"]).astype(np.int64)

    # ---- fold BN into the linears ----
    W1p, b1p = _fold_bn(
        np.asarray(inputs["W1"]), np.asarray(inputs["b1"]),
        np.asarray(inputs["g1"]), np.asarray(inputs["be1"]),
        np.asarray(inputs["m1"]), np.asarray(inputs["v1"]),
    )
    W2p, b2p = _fold_bn(
        np.asarray(inputs["W2"]), np.asarray(inputs["b2"]),
        np.asarray(inputs["g2"]), np.asarray(inputs["be2"]),
        np.asarray(inputs["m2"]), np.asarray(inputs["v2"]),
    )
    W3p, b3p = _fold_bn(
        np.asarray(inputs["W3"]), np.asarray(inputs["b3"]),
        np.asarray(inputs["g3"]), np.asarray(inputs["be3"]),
        np.asarray(inputs["m3"]), np.asarray(inputs["v3"]),
    )
    Wop, bop = _fold_bn(
        np.asarray(inputs["Wo"]), np.asarray(inputs["bo"]),
        np.asarray(inputs["go"]), np.asarray(inputs["beo"]),
        np.asarray(inputs["mo"]), np.asarray(inputs["vo"]),
    )

    # ---- whole-segment sharding by sorted-width round-robin rank ----
    counts = np.bincount(batch, minlength=NSEG).astype(np.int64)
    assert np.all(batch[:-1] <= batch[1:]), "batch must be sorted"
    order = np.argsort(-counts, kind="stable")  # segment ids, width desc
    slot_w = np.maximum(counts[order[::NCORES][:S]], 1)  # width of rank 8k
    tiles, cols = _plan_tiles(slot_w)

    key = (cols, tuple(slot_w.tolist()))
    if key not in _compiled_cache:
        _compiled_cache[key] = _build_program(tiles, cols)
    nc = _compiled_cache[key]

    # column start / padded width of each slot
    slot_col = np.zeros(S, dtype=np.int64)
    slot_wt = np.zeros(S, dtype=np.int64)
    for k0, d, wt, col0 in tiles:
        for j in range(d):
            slot_col[k0 + j] = col0 + j * wt
        slot_wt[k0 : k0 + d] = wt

    starts = np.searchsorted(batch, np.arange(NSEG), side="left")
    ends = np.searchsorted(batch, np.arange(NSEG), side="right")

    bf = ml_dtypes.bfloat16
    in_maps = []
    for c in range(NCORES):
        segs = order[np.arange(S) * NCORES + c]  # this core's segment ids
        ccnt = counts[segs]
        src = np.full(cols, -1, dtype=np.int64)
        for k in range(S):
            s = segs[k]
            cnt = int(ccnt[k])
            c0 = slot_col[k]
            if cnt:
                src[c0 : c0 + cnt] = np.arange(starts[s], ends[s])
                # pads duplicate the first member (exact for max; sum fixed
                # on device via the npad correction)
                src[c0 + cnt : c0 + slot_wt[k]] = starts[s]
        real = src >= 0
        xTc = np.zeros((C, cols), dtype=bf)
        xTc[:, real] = x[src[real]].T.astype(bf)
        npadc = np.where(ccnt > 0, -(slot_wt - ccnt), 0).astype(np.float32)
        recipc = (1.0 / np.maximum(ccnt, 1.0)).astype(np.float32)
        in_maps.append(
            dict(
                xT=xTc,
                w1=np.ascontiguousarray(W1p.T).astype(bf),
                w2=np.ascontiguousarray(W2p.T).astype(bf),
                w3=np.ascontiguousarray(W3p.T).astype(bf),
                b1=np.ascontiguousarray(b1p[:, None]),
                b2=np.ascontiguousarray(b2p[:, None]),
                b3=np.ascontiguousarray(b3p[:, None]),
                npad=np.ascontiguousarray(
                    np.broadcast_to(npadc[None, :], (H, S))
                ),
                wsum=np.ascontiguousarray(Wop[:, 0:H].T),
                wmax=np.ascontiguousarray(Wop[:, H : 2 * H].T),
                wmean=np.ascontiguousarray(Wop[:, 2 * H : 3 * H].T),
                bo=np.ascontiguousarray(bop[None, :]),
                recip=np.ascontiguousarray(recipc.reshape(S // H, H).T),
            )
        )

    ncores_run = int(os.environ.get("KERNEL_NCORES", str(NCORES)))
    res = bass_utils.run_bass_kernel_spmd(
        nc,
        in_maps[:ncores_run],
        core_ids=list(range(ncores_run)),
        trace=bool(int(os.environ.get("KERNEL_TRACE", "0"))),
        tmpdir=os.environ.get("KERNEL_TRACE_DIR") or None,
    )
    kernel.last_results = res

    out_full = np.zeros((NSEG, O), dtype=np.float32)
    ranks = np.arange(S)
    for c in range(ncores_run):
        out_full[order[ranks * NCORES + c]] = res.results[c]["out"]
    # empty segments: agg == 0, so the output row is just the folded bias
    out_full[counts == 0] = bop
    return out_full


# revision 25
# speedup vs baseline: 1.3699x; 1.0003x over previous
"""DeepSets segment-reduce kernel for 8x Trainium2 NeuronCores.

Strategy (all shapes hardcoded for N=500000, C=H=128, O=64, NSEG=2048):
  - Transposed activation layout: features on SBUF partitions, nodes on the
    free axis, so segment reductions are free-axis operations.
  - Whole-segment sharding: every segment is assigned entirely to one core,
    round-robin by global sorted-width rank.  All 8 cores share an identical
    compile-time slot/tile geometry (SPMD-safe); per-core padding is small.
    No collective is needed - the host gather is the unshard.
  - Encoder BN is folded into the linear weights; each layer is relu(W'x+b').
  - bf16 activations/weights halve DMA bytes and SBUF traffic; PSUM stays
    fp32.  Error budget (2e-2) dwarfs bf16 noise.
  - Pad columns DUPLICATE the slot's first member, so the segment max is
    exact for free; the inflated segment sum is fixed by one rank-1
    subtraction per tile (npad * h3[first]).  Empty segments are patched on
    the host (their output row is the folded bias).
  - Per-slot segment sum AND max are single-instruction DVE tensor_scalar
    ops with accum_out (fold op = op1) reading post-relu bf16 h3 from SBUF.
  - Tiles are processed in pairs: one DMA per pair (2KB/partition lines),
    LDWEIGHTS amortized over both tiles, and relu3 is one ScalarE
    instruction over a 2-bank PSUM pair.
  - Final projection out = [sum|max|mean] @ Wo'.T + bo' runs per core on its
    own 256 segments; mean rides the sum through the Wmean block with a
    per-slot reciprocal row-scale.
"""

import os
import sys

import numpy as np

if "/opt/trn_rl_repo" not in sys.path:
    sys.path.insert(0, "/opt/trn_rl_repo")

import ml_dtypes

import concourse.bacc as bacc
import concourse.mybir as mybir
import concourse.tile as tile
from concourse import bass_utils

EPS = 1e-5
NSEG = 2048
NCORES = 8
C = 128
H = 128
O = 64
S = NSEG // NCORES  # segment slots per core (256)
MAX_TILE = 512  # PSUM bank limit (fp32)

_compiled_cache = {}


def _fold_bn(W, b, g, be, m, v):
    a = g / np.sqrt(v + EPS)
    Wp = W * a[:, None]
    bp = (b - m) * a + be
    return Wp.astype(np.float32), bp.astype(np.float32)


def _plan_tiles(slot_w):
    """Greedy-pack slots (widths descending) into tiles of <=MAX_TILE cols.

    Returns list of (slot_start, n_slots, padded_width, col_start) and the
    total padded column count.
    """
    tiles = []
    col = 0
    k = 0
    n = len(slot_w)
    while k < n:
        # multiples of 4 so bf16 half-slot views stay 4B-aligned (2x_1P)
        wt = (int(slot_w[k]) + 3) & ~3
        assert 0 < wt <= MAX_TILE, f"slot width {wt} unsupported"
        d = min(MAX_TILE // wt, n - k)
        tiles.append((k, d, wt, col))
        col += d * wt
        k += d
    return tiles, col


def _build_program(tiles, cols):
    """Emit the Bass/Tile program shared by all 8 cores."""
    nc = bacc.Bacc(
        "TRN2",
        target_bir_lowering=False,
        debug=False,
        num_devices=NCORES,
    )
    f32 = mybir.dt.float32
    bf16 = mybir.dt.bfloat16

    xT = nc.dram_tensor("xT", [C, cols], bf16, kind="ExternalInput").ap()
    w1 = nc.dram_tensor("w1", [C, H], bf16, kind="ExternalInput").ap()
    w2 = nc.dram_tensor("w2", [H, H], bf16, kind="ExternalInput").ap()
    w3 = nc.dram_tensor("w3", [H, H], bf16, kind="ExternalInput").ap()
    b1 = nc.dram_tensor("b1", [H, 1], f32, kind="ExternalInput").ap()
    b2 = nc.dram_tensor("b2", [H, 1], f32, kind="ExternalInput").ap()
    b3 = nc.dram_tensor("b3", [H, 1], f32, kind="ExternalInput").ap()
    npad = nc.dram_tensor("npad", [H, S], f32, kind="ExternalInput").ap()
    wsum = nc.dram_tensor("wsum", [H, O], f32, kind="ExternalInput").ap()
    wmax = nc.dram_tensor("wmax", [H, O], f32, kind="ExternalInput").ap()
    wmean = nc.dram_tensor("wmean", [H, O], f32, kind="ExternalInput").ap()
    bo = nc.dram_tensor("bo", [1, O], f32, kind="ExternalInput").ap()
    # column ch holds the reciprocals for segment chunk ch (128 slots each)
    recip = nc.dram_tensor("recip", [H, S // H], f32, kind="ExternalInput").ap()
    out = nc.dram_tensor("out", [S, O], f32, kind="ExternalOutput").ap()

    relu = mybir.ActivationFunctionType.Relu
    add = mybir.AluOpType.add
    amax = mybir.AluOpType.max
    asub = mybir.AluOpType.subtract
    amult = mybir.AluOpType.mult

    PAIR = 2 * MAX_TILE

    with tile.TileContext(nc) as tc:
        with (
            tc.tile_pool(name="const", bufs=1) as cpool,
            tc.tile_pool(name="xin", bufs=10) as xpool,
            tc.tile_pool(name="h1", bufs=8) as h1pool,
            tc.tile_pool(name="h2", bufs=10) as h2pool,
            tc.tile_pool(name="h3", bufs=8) as h3pool,
            tc.tile_pool(name="scr", bufs=6) as scrpool,
            tc.tile_pool(name="acc", bufs=1) as accpool,
            tc.tile_pool(name="ps1", bufs=2, space="PSUM") as ps1,
            tc.tile_pool(name="ps2", bufs=2, space="PSUM") as ps2,
            tc.tile_pool(name="ps3", bufs=1, space="PSUM") as ps3,
        ):
            w1s = cpool.tile([C, H], bf16, tag="w1")
            w2s = cpool.tile([H, H], bf16, tag="w2")
            w3s = cpool.tile([H, H], bf16, tag="w3")
            b1s = cpool.tile([H, 1], f32, tag="b1")
            b2s = cpool.tile([H, 1], f32, tag="b2")
            b3s = cpool.tile([H, 1], f32, tag="b3")
            npads = cpool.tile([H, S], f32, tag="npad")
            wsums = cpool.tile([H, O], f32, tag="wsum")
            wmaxs = cpool.tile([H, O], f32, tag="wmax")
            wmeans = cpool.tile([H, O], f32, tag="wmean")
            bos = cpool.tile([1, O], f32, tag="bo")
            recs = cpool.tile([H, S // H], f32, tag="recip")
            ones = cpool.tile([1, H], f32, tag="ones")

            nc.sync.dma_start(w1s[:], w1)
            nc.sync.dma_start(w2s[:], w2)
            nc.sync.dma_start(w3s[:], w3)
            nc.sync.dma_start(b1s[:], b1)
            nc.sync.dma_start(b2s[:], b2)
            nc.sync.dma_start(b3s[:], b3)
            nc.sync.dma_start(npads[:], npad)
            nc.sync.dma_start(wsums[:], wsum)
            nc.sync.dma_start(wmaxs[:], wmax)
            nc.sync.dma_start(wmeans[:], wmean)
            nc.sync.dma_start(bos[:], bo)
            nc.sync.dma_start(recs[:], recip)
            nc.vector.memset(ones[:], 1.0)

            # Persistent per-slot partials (post-relu sums and maxes) plus
            # the duplicated-pad repair term (-npad_k * h3[:, first_k]).
            sumP = accpool.tile([H, S], f32, tag="sumP")
            maxP = accpool.tile([H, S], f32, tag="maxP")
            padC = accpool.tile([H, S], f32, tag="padC")

            pairs = [tiles[i : i + 2] for i in range(0, len(tiles), 2)]

            pi = 0
            for pair in pairs:
                pcols = sum(d * wt for (_, d, wt, _) in pair)
                pcol0 = pair[0][3]
                xt = xpool.tile([C, PAIR], bf16, tag="xt")
                nc.sync.dma_start(xt[:, :pcols], xT[:, pcol0 : pcol0 + pcols])

                # Layer 1 into a 2-bank pair; one fused relu over both tiles.
                p1 = ps1.tile([H, PAIR], f32, tag="p1")
                for i, (k0, d, wt, col0) in enumerate(pair):
                    tcols = d * wt
                    o0 = col0 - pcol0
                    nc.tensor.matmul(
                        p1[:, i * MAX_TILE : i * MAX_TILE + tcols],
                        w1s[:], xt[:, o0 : o0 + tcols],
                    )
                span1 = (len(pair) - 1) * MAX_TILE + pair[-1][1] * pair[-1][2]
                h1 = h1pool.tile([H, PAIR], bf16, tag="h1")
                nc.scalar.activation(
                    h1[:, :span1], p1[:, :span1], relu, bias=b1s[:]
                )

                p2s, h2s = [], []
                for i, (k0, d, wt, col0) in enumerate(pair):
                    tcols = d * wt
                    p2 = ps2.tile([H, MAX_TILE], f32, tag="p2")
                    nc.tensor.matmul(
                        p2[:, :tcols], w2s[:],
                        h1[:, i * MAX_TILE : i * MAX_TILE + tcols],
                    )
                    p2s.append(p2)
                for i, ((k0, d, wt, col0), p2) in enumerate(zip(pair, p2s)):
                    tcols = d * wt
                    h2 = h2pool.tile([H, MAX_TILE], bf16, tag="h2")
                    if (pi + i) % 16 < 9:  # 9/16 of relu2 on ScalarE
                        nc.scalar.activation(
                            h2[:, :tcols], p2[:, :tcols], relu, bias=b2s[:]
                        )
                    else:
                        nc.vector.tensor_scalar(
                            h2[:, :tcols], p2[:, :tcols], b2s[:], 0.0,
                            op0=add, op1=amax,
                        )
                    h2s.append(h2)

                # Layer 3 into a 2-bank PSUM pair; one fused relu over both.
                p3 = ps3.tile([H, PAIR], f32, tag="p3")
                offs = []
                for i, ((k0, d, wt, col0), h2) in enumerate(zip(pair, h2s)):
                    tcols = d * wt
                    o3 = i * MAX_TILE  # tile i at its own bank
                    nc.tensor.matmul(
                        p3[:, o3 : o3 + tcols], w3s[:], h2[:, :tcols]
                    )
                    offs.append(o3)
                span = offs[-1] + pair[-1][1] * pair[-1][2]
                h3 = h3pool.tile([H, PAIR], bf16, tag="h3")
                nc.scalar.activation(
                    h3[:, :span], p3[:, :span], relu, bias=b3s[:]
                )

                # Segment reduces: one DVE tree-halving level (2x_1P on bf16
                # halves of each slot) then a half-width 1x reduce per tile.
                tm = scrpool.tile([H, MAX_TILE], bf16, tag="tm")
                ts = scrpool.tile([H, MAX_TILE], bf16, tag="ts")
                for (k0, d, wt, col0), o3 in zip(pair, offs):
                    hw = wt // 2
                    h3v = h3[:, o3 : o3 + d * wt].rearrange(
                        "p (d w) -> p d w", d=d
                    )
                    lo = h3v[:, :, :hw]
                    hi = h3v[:, :, hw:]
                    tmv = tm[:, o3 // 2 : o3 // 2 + d * hw].rearrange(
                        "p (d w) -> p d w", d=d
                    )
                    tsv = ts[:, o3 // 2 : o3 // 2 + d * hw].rearrange(
                        "p (d w) -> p d w", d=d
                    )
                    nc.vector.tensor_tensor(tmv, lo, hi, op=amax)
                    nc.vector.tensor_tensor(tsv, lo, hi, op=add)
                    nc.vector.reduce_max(
                        maxP[:, k0 : k0 + d], tmv, axis=mybir.AxisListType.X
                    )
                    nc.vector.reduce_sum(
                        sumP[:, k0 : k0 + d], tsv, axis=mybir.AxisListType.X
                    )
                    # pad repair term: padC[:,k] = -npad_k * h3[:,first_k]
                    # (npads arrives negated; the epilogue adds padC @ W)
                    h3f = h3v[:, :, 0:1]
                    npv = npads[:, k0 : k0 + d].rearrange(
                        "p (d w) -> p d w", w=1
                    )
                    pcv = padC[:, k0 : k0 + d].rearrange(
                        "p (d w) -> p d w", w=1
                    )
                    nc.vector.tensor_tensor(pcv, h3f, npv, op=amult)
                pi += len(pair)

            # ---- epilogue: out[k, :] = sum_k @ Wsum + max_k @ Wmax
            #                + (sum_k * recip_k) @ Wmean + bo ----
            for ch in range(S // H):  # 2 chunks of 128 segments
                sl = slice(ch * H, (ch + 1) * H)
                pot = ps1.tile([H, PAIR], f32, tag="p1")
                po = pot[:, :O]
                nc.tensor.matmul(po, sumP[:, sl], wsums[:], start=True, stop=False)
                nc.tensor.matmul(po, padC[:, sl], wsums[:], start=False, stop=False)
                nc.tensor.matmul(po, maxP[:, sl], wmaxs[:], start=False, stop=False)
                nc.tensor.matmul(po, ones[:], bos[:], start=False, stop=True)

                pmt = ps2.tile([H, MAX_TILE], f32, tag="p2")
                pm = pmt[:, :O]
                nc.tensor.matmul(pm, sumP[:, sl], wmeans[:], start=True, stop=False)
                nc.tensor.matmul(pm, padC[:, sl], wmeans[:], start=False, stop=True)

                om = h1pool.tile([H, O], f32, tag="om")
                nc.vector.tensor_scalar_mul(om[:], pm, recs[:, ch : ch + 1])
                ot = h2pool.tile([H, O], f32, tag="ot")
                nc.vector.tensor_tensor(ot[:], po, om[:], op=add)
                nc.sync.dma_start(out[sl, :], ot[:])

    nc.compile()
    return nc


def kernel(**inputs):
    x = np.ascontiguousarray(np.asarray(inputs["x"], dtype=np.float32))
    batch = np.asarray(inputs["batch"]).astype(np.int64)

    # ---- fold BN into the linears ----
    W1p, b1p = _fold_bn(
        np.asarray(inputs["W1"]), np.asarray(inputs["b1"]),
        np.asarray(inputs["g1"]), np.asarray(inputs["be1"]),
        np.asarray(inputs["m1"]), np.asarray(inputs["v1"]),
    )
    W2p, b2p = _fold_bn(
        np.asarray(inputs["W2"]), np.asarray(inputs["b2"]),
        np.asarray(inputs["g2"]), np.asarray(inputs["be2"]),
        np.asarray(inputs["m2"]), np.asarray(inputs["v2"]),
    )
    W3p, b3p = _fold_bn(
        np.asarray(inputs["W3"]), np.asarray(inputs["b3"]),
        np.asarray(inputs["g3"]), np.asarray(inputs["be3"]),
        np.asarray(inputs["m3"]), np.asarray(inputs["v3"]),
    )
    Wop, bop = _fold_bn(
        np.asarray(inputs["Wo"]), np.asarray(inputs["bo"]),
        np.asarray(inputs["go"]), np.asarray(inputs["beo"]),
        np.asarray(inputs["mo"]), np.asarray(inputs["vo"]),
    )

    # ---- whole-segment sharding by sorted-width round-robin rank ----
    counts = np.bincount(batch, minlength=NSEG).astype(np.int64)
    assert np.all(batch[:-1] <= batch[1:]), "batch must be sorted"
    order = np.argsort(-counts, kind="stable")  # segment ids, width desc
    slot_w = np.maximum(counts[order[::NCORES][:S]], 1)  # width of rank 8k
    tiles, cols = _plan_tiles(slot_w)

    key = (cols, tuple(slot_w.tolist()))
    if key not in _compiled_cache:
        _compiled_cache[key] = _build_program(tiles, cols)
    nc = _compiled_cache[key]

    # column start / padded width of each slot
    slot_col = np.zeros(S, dtype=np.int64)
    slot_wt = np.zeros(S, dtype=np.int64)
    for k0, d, wt, col0 in tiles:
        for j in range(d):
            slot_col[k0 + j] = col0 + j * wt
        slot_wt[k0 : k0 + d] = wt

    starts = np.searchsorted(batch, np.arange(NSEG), side="left")
    ends = np.searchsorted(batch, np.arange(NSEG), side="right")

    bf = ml_dtypes.bfloat16
    in_maps = []
    for c in range(NCORES):
        segs = order[np.arange(S) * NCORES + c]  # this core's segment ids
        ccnt = counts[segs]
        src = np.full(cols, -1, dtype=np.int64)
        for k in range(S):
            s = segs[k]
            cnt = int(ccnt[k])
            c0 = slot_col[k]
            if cnt:
                src[c0 : c0 + cnt] = np.arange(starts[s], ends[s])
                # pads duplicate the first member (exact for max; sum fixed
                # on device via the npad correction)
                src[c0 + cnt : c0 + slot_wt[k]] = starts[s]
        real = src >= 0
        xTc = np.zeros((C, cols), dtype=bf)
        xTc[:, real] = x[src[real]].T.astype(bf)
        npadc = np.where(ccnt > 0, -(slot_wt - ccnt), 0).astype(np.float32)
        recipc = (1.0 / np.maximum(ccnt, 1.0)).astype(np.float32)
        in_maps.append(
            dict(
                xT=xTc,
                w1=np.ascontiguousarray(W1p.T).astype(bf),
                w2=np.ascontiguousarray(W2p.T).astype(bf),
                w3=np.ascontiguousarray(W3p.T).astype(bf),
                b1=np.ascontiguousarray(b1p[:, None]),
                b2=np.ascontiguousarray(b2p[:, None]),
                b3=np.ascontiguousarray(b3p[:, None]),
                npad=np.ascontiguousarray(
                    np.broadcast_to(npadc[None, :], (H, S))
                ),
                wsum=np.ascontiguousarray(Wop[:, 0:H].T),
                wmax=np.ascontiguousarray(Wop[:, H : 2 * H].T),
                wmean=np.ascontiguousarray(Wop[:, 2 * H : 3 * H].T),
                bo=np.ascontiguousarray(bop[None, :]),
                recip=np.ascontiguousarray(recipc.reshape(S // H, H).T),
            )
        )

    ncores_run = int(os.environ.get("KERNEL_NCORES", str(NCORES)))
    res = bass_utils.run_bass_kernel_spmd(
        nc,
        in_maps[:ncores_run],
        core_ids=list(range(ncores_run)),
        trace=bool(int(os.environ.get("KERNEL_TRACE", "0"))),
        tmpdir=os.environ.get("KERNEL_TRACE_DIR") or None,
    )
    kernel.last_results = res

    out_full = np.zeros((NSEG, O), dtype=np.float32)
    ranks = np.arange(S)
    for c in range(ncores_run):
        out_full[order[ranks * NCORES + c]] = res.results[c]["out"]
    # empty segments: agg == 0, so the output row is just the folded bias
    out_full[counts == 0] = bop
    return out_full


# revision 29
# speedup vs baseline: 1.3706x; 1.0006x over previous
"""DeepSets segment-reduce kernel for 8x Trainium2 NeuronCores.

Strategy (all shapes hardcoded for N=500000, C=H=128, O=64, NSEG=2048):
  - Transposed activation layout: features on SBUF partitions, nodes on the
    free axis, so segment reductions are free-axis operations.
  - Whole-segment sharding: every segment is assigned entirely to one core,
    round-robin by global sorted-width rank.  All 8 cores share an identical
    compile-time slot/tile geometry (SPMD-safe); per-core padding is small.
    No collective is needed - the host gather is the unshard.
  - Encoder BN is folded into the linear weights; each layer is relu(W'x+b').
  - bf16 activations/weights halve DMA bytes and SBUF traffic; PSUM stays
    fp32.  Error budget (2e-2) dwarfs bf16 noise.
  - Pad columns DUPLICATE the slot's first member, so the segment max is
    exact for free; the inflated segment sum is fixed by one rank-1
    subtraction per tile (npad * h3[first]).  Empty segments are patched on
    the host (their output row is the folded bias).
  - Per-slot segment sum AND max are single-instruction DVE tensor_scalar
    ops with accum_out (fold op = op1) reading post-relu bf16 h3 from SBUF.
  - Tiles are processed in pairs: one DMA per pair (2KB/partition lines),
    LDWEIGHTS amortized over both tiles, and relu3 is one ScalarE
    instruction over a 2-bank PSUM pair.
  - Final projection out = [sum|max|mean] @ Wo'.T + bo' runs per core on its
    own 256 segments; mean rides the sum through the Wmean block with a
    per-slot reciprocal row-scale.
"""

import os
import sys

import numpy as np

if "/opt/trn_rl_repo" not in sys.path:
    sys.path.insert(0, "/opt/trn_rl_repo")

import ml_dtypes

import concourse.bacc as bacc
import concourse.mybir as mybir
import concourse.tile as tile
from concourse import bass_utils

EPS = 1e-5
NSEG = 2048
NCORES = 8
C = 128
H = 128
O = 64
S = NSEG // NCORES  # segment slots per core (256)
MAX_TILE = 512  # PSUM bank limit (fp32)

_compiled_cache = {}


def _fold_bn(W, b, g, be, m, v):
    a = g / np.sqrt(v + EPS)
    Wp = W * a[:, None]
    bp = (b - m) * a + be
    return Wp.astype(np.float32), bp.astype(np.float32)


def _plan_tiles(slot_w):
    """Greedy-pack slots (widths descending) into tiles of <=MAX_TILE cols.

    Returns list of (slot_start, n_slots, padded_width, col_start) and the
    total padded column count.
    """
    tiles = []
    col = 0
    k = 0
    n = len(slot_w)
    while k < n:
        # multiples of 4 so bf16 half-slot views stay 4B-aligned (2x_1P)
        wt = (int(slot_w[k]) + 3) & ~3
        assert 0 < wt <= MAX_TILE, f"slot width {wt} unsupported"
        d = min(MAX_TILE // wt, n - k)
        tiles.append((k, d, wt, col))
        col += d * wt
        k += d
    return tiles, col


def _build_program(tiles, cols):
    """Emit the Bass/Tile program shared by all 8 cores."""
    nc = bacc.Bacc(
        "TRN2",
        target_bir_lowering=False,
        debug=False,
        num_devices=NCORES,
    )
    f32 = mybir.dt.float32
    bf16 = mybir.dt.bfloat16

    xT = nc.dram_tensor("xT", [C, cols], bf16, kind="ExternalInput").ap()
    w1 = nc.dram_tensor("w1", [C, H], bf16, kind="ExternalInput").ap()
    w2 = nc.dram_tensor("w2", [H, H], bf16, kind="ExternalInput").ap()
    w3 = nc.dram_tensor("w3", [H, H], bf16, kind="ExternalInput").ap()
    b1 = nc.dram_tensor("b1", [H, 1], f32, kind="ExternalInput").ap()
    b2 = nc.dram_tensor("b2", [H, 1], f32, kind="ExternalInput").ap()
    b3 = nc.dram_tensor("b3", [H, 1], f32, kind="ExternalInput").ap()
    npad = nc.dram_tensor("npad", [H, S], f32, kind="ExternalInput").ap()
    wsum = nc.dram_tensor("wsum", [H, O], f32, kind="ExternalInput").ap()
    wmax = nc.dram_tensor("wmax", [H, O], f32, kind="ExternalInput").ap()
    wmean = nc.dram_tensor("wmean", [H, O], f32, kind="ExternalInput").ap()
    bo = nc.dram_tensor("bo", [1, O], f32, kind="ExternalInput").ap()
    # column ch holds the reciprocals for segment chunk ch (128 slots each)
    recip = nc.dram_tensor("recip", [H, S // H], f32, kind="ExternalInput").ap()
    out = nc.dram_tensor("out", [S, O], f32, kind="ExternalOutput").ap()

    relu = mybir.ActivationFunctionType.Relu
    add = mybir.AluOpType.add
    amax = mybir.AluOpType.max
    asub = mybir.AluOpType.subtract
    amult = mybir.AluOpType.mult

    PAIR = 2 * MAX_TILE

    with tile.TileContext(nc) as tc:
        with (
            tc.tile_pool(name="const", bufs=1) as cpool,
            tc.tile_pool(name="xin", bufs=10) as xpool,
            tc.tile_pool(name="h1", bufs=8) as h1pool,
            tc.tile_pool(name="h2", bufs=10) as h2pool,
            tc.tile_pool(name="h3", bufs=8) as h3pool,
            tc.tile_pool(name="scr", bufs=6) as scrpool,
            tc.tile_pool(name="acc", bufs=1) as accpool,
            tc.tile_pool(name="ps1", bufs=2, space="PSUM") as ps1,
            tc.tile_pool(name="ps2", bufs=2, space="PSUM") as ps2,
            tc.tile_pool(name="ps3", bufs=1, space="PSUM") as ps3,
        ):
            w1s = cpool.tile([C, H], bf16, tag="w1")
            w2s = cpool.tile([H, H], bf16, tag="w2")
            w3s = cpool.tile([H, H], bf16, tag="w3")
            b1s = cpool.tile([H, 1], f32, tag="b1")
            b2s = cpool.tile([H, 1], f32, tag="b2")
            b3s = cpool.tile([H, 1], f32, tag="b3")
            npads = cpool.tile([H, S], f32, tag="npad")
            wsums = cpool.tile([H, O], f32, tag="wsum")
            wmaxs = cpool.tile([H, O], f32, tag="wmax")
            wmeans = cpool.tile([H, O], f32, tag="wmean")
            bos = cpool.tile([1, O], f32, tag="bo")
            recs = cpool.tile([H, S // H], f32, tag="recip")
            ones = cpool.tile([1, H], f32, tag="ones")

            # only the encoder weights gate the first tiles; epilogue-only
            # constants are DMA'd after the loop body is queued
            nc.sync.dma_start(w1s[:], w1)
            nc.sync.dma_start(w2s[:], w2)
            nc.sync.dma_start(w3s[:], w3)
            nc.sync.dma_start(b1s[:], b1)
            nc.sync.dma_start(b2s[:], b2)
            nc.sync.dma_start(b3s[:], b3)
            nc.sync.dma_start(npads[:], npad)
            nc.vector.memset(ones[:], 1.0)

            # Persistent per-slot partials (post-relu sums and maxes) plus
            # the duplicated-pad repair term (-npad_k * h3[:, first_k]).
            sumP = accpool.tile([H, S], f32, tag="sumP")
            maxP = accpool.tile([H, S], f32, tag="maxP")
            padC = accpool.tile([H, S], f32, tag="padC")

            pairs = [tiles[i : i + 2] for i in range(0, len(tiles), 2)]

            pi = 0
            for pair in pairs:
                pcols = sum(d * wt for (_, d, wt, _) in pair)
                pcol0 = pair[0][3]
                xt = xpool.tile([C, PAIR], bf16, tag="xt")
                nc.sync.dma_start(xt[:, :pcols], xT[:, pcol0 : pcol0 + pcols])
                if pi == 4:  # pipeline is rolling; fetch epilogue constants
                    nc.sync.dma_start(wsums[:], wsum)
                    nc.sync.dma_start(wmaxs[:], wmax)
                    nc.sync.dma_start(wmeans[:], wmean)
                    nc.sync.dma_start(bos[:], bo)
                    nc.sync.dma_start(recs[:], recip)

                # Layer 1 into a 2-bank pair; one fused relu over both tiles.
                p1 = ps1.tile([H, PAIR], f32, tag="p1")
                for i, (k0, d, wt, col0) in enumerate(pair):
                    tcols = d * wt
                    o0 = col0 - pcol0
                    nc.tensor.matmul(
                        p1[:, i * MAX_TILE : i * MAX_TILE + tcols],
                        w1s[:], xt[:, o0 : o0 + tcols],
                    )
                span1 = (len(pair) - 1) * MAX_TILE + pair[-1][1] * pair[-1][2]
                h1 = h1pool.tile([H, PAIR], bf16, tag="h1")
                nc.scalar.activation(
                    h1[:, :span1], p1[:, :span1], relu, bias=b1s[:]
                )

                p2s, h2s = [], []
                for i, (k0, d, wt, col0) in enumerate(pair):
                    tcols = d * wt
                    p2 = ps2.tile([H, MAX_TILE], f32, tag="p2")
                    nc.tensor.matmul(
                        p2[:, :tcols], w2s[:],
                        h1[:, i * MAX_TILE : i * MAX_TILE + tcols],
                    )
                    p2s.append(p2)
                for i, ((k0, d, wt, col0), p2) in enumerate(zip(pair, p2s)):
                    tcols = d * wt
                    h2 = h2pool.tile([H, MAX_TILE], bf16, tag="h2")
                    if (pi + i) % 16 < 9:  # 9/16 of relu2 on ScalarE
                        nc.scalar.activation(
                            h2[:, :tcols], p2[:, :tcols], relu, bias=b2s[:]
                        )
                    else:
                        nc.vector.tensor_scalar(
                            h2[:, :tcols], p2[:, :tcols], b2s[:], 0.0,
                            op0=add, op1=amax,
                        )
                    h2s.append(h2)

                # Layer 3 into a 2-bank PSUM pair; one fused relu over both.
                p3 = ps3.tile([H, PAIR], f32, tag="p3")
                offs = []
                for i, ((k0, d, wt, col0), h2) in enumerate(zip(pair, h2s)):
                    tcols = d * wt
                    o3 = i * MAX_TILE  # tile i at its own bank
                    nc.tensor.matmul(
                        p3[:, o3 : o3 + tcols], w3s[:], h2[:, :tcols]
                    )
                    offs.append(o3)
                span = offs[-1] + pair[-1][1] * pair[-1][2]
                h3 = h3pool.tile([H, PAIR], bf16, tag="h3")
                nc.scalar.activation(
                    h3[:, :span], p3[:, :span], relu, bias=b3s[:]
                )

                # Segment reduces: one DVE tree-halving level (2x_1P on bf16
                # halves of each slot) then a half-width 1x reduce per tile.
                tm = scrpool.tile([H, MAX_TILE], bf16, tag="tm")
                ts = scrpool.tile([H, MAX_TILE], bf16, tag="ts")
                for (k0, d, wt, col0), o3 in zip(pair, offs):
                    hw = wt // 2
                    h3v = h3[:, o3 : o3 + d * wt].rearrange(
                        "p (d w) -> p d w", d=d
                    )
                    lo = h3v[:, :, :hw]
                    hi = h3v[:, :, hw:]
                    tmv = tm[:, o3 // 2 : o3 // 2 + d * hw].rearrange(
                        "p (d w) -> p d w", d=d
                    )
                    tsv = ts[:, o3 // 2 : o3 // 2 + d * hw].rearrange(
                        "p (d w) -> p d w", d=d
                    )
                    nc.vector.tensor_tensor(tmv, lo, hi, op=amax)
                    nc.vector.tensor_tensor(tsv, lo, hi, op=add)
                    nc.vector.reduce_max(
                        maxP[:, k0 : k0 + d], tmv, axis=mybir.AxisListType.X
                    )
                    nc.vector.reduce_sum(
                        sumP[:, k0 : k0 + d], tsv, axis=mybir.AxisListType.X
                    )
                    # pad repair term: padC[:,k] = -npad_k * h3[:,first_k]
                    # (npads arrives negated; the epilogue adds padC @ W)
                    h3f = h3v[:, :, 0:1]
                    npv = npads[:, k0 : k0 + d].rearrange(
                        "p (d w) -> p d w", w=1
                    )
                    pcv = padC[:, k0 : k0 + d].rearrange(
                        "p (d w) -> p d w", w=1
                    )
                    nc.vector.tensor_tensor(pcv, h3f, npv, op=amult)
                pi += len(pair)

            # ---- epilogue: out[k, :] = sum_k @ Wsum + max_k @ Wmax
            #                + (sum_k * recip_k) @ Wmean + bo ----
            for ch in range(S // H):  # 2 chunks of 128 segments
                sl = slice(ch * H, (ch + 1) * H)
                pot = ps1.tile([H, PAIR], f32, tag="p1")
                po = pot[:, :O]
                nc.tensor.matmul(po, sumP[:, sl], wsums[:], start=True, stop=False)
                nc.tensor.matmul(po, padC[:, sl], wsums[:], start=False, stop=False)
                nc.tensor.matmul(po, maxP[:, sl], wmaxs[:], start=False, stop=False)
                nc.tensor.matmul(po, ones[:], bos[:], start=False, stop=True)

                pmt = ps2.tile([H, MAX_TILE], f32, tag="p2")
                pm = pmt[:, :O]
                nc.tensor.matmul(pm, sumP[:, sl], wmeans[:], start=True, stop=False)
                nc.tensor.matmul(pm, padC[:, sl], wmeans[:], start=False, stop=True)

                om = h1pool.tile([H, O], f32, tag="om")
                nc.vector.tensor_scalar_mul(om[:], pm, recs[:, ch : ch + 1])
                ot = h2pool.tile([H, O], f32, tag="ot")
                nc.vector.tensor_tensor(ot[:], po, om[:], op=add)
                nc.sync.dma_start(out[sl, :], ot[:])

    nc.compile()
    return nc


def kernel(**inputs):
    x = np.ascontiguousarray(np.asarray(inputs["x"], dtype=np.float32))
    batch = np.asarray(inputs["batch"]).astype(np.int64)

    # ---- fold BN into the linears ----
    W1p, b1p = _fold_bn(
        np.asarray(inputs["W1"]), np.asarray(inputs["b1"]),
        np.asarray(inputs["g1"]), np.asarray(inputs["be1"]),
        np.asarray(inputs["m1"]), np.asarray(inputs["v1"]),
    )
    W2p, b2p = _fold_bn(
        np.asarray(inputs["W2"]), np.asarray(inputs["b2"]),
        np.asarray(inputs["g2"]), np.asarray(inputs["be2"]),
        np.asarray(inputs["m2"]), np.asarray(inputs["v2"]),
    )
    W3p, b3p = _fold_bn(
        np.asarray(inputs["W3"]), np.asarray(inputs["b3"]),
        np.asarray(inputs["g3"]), np.asarray(inputs["be3"]),
        np.asarray(inputs["m3"]), np.asarray(inputs["v3"]),
    )
    Wop, bop = _fold_bn(
        np.asarray(inputs["Wo"]), np.asarray(inputs["bo"]),
        np.asarray(inputs["go"]), np.asarray(inputs["beo"]),
        np.asarray(inputs["mo"]), np.asarray(inputs["vo"]),
    )

    # ---- whole-segment sharding by sorted-width round-robin rank ----
    counts = np.bincount(batch, minlength=NSEG).astype(np.int64)
    assert np.all(batch[:-1] <= batch[1:]), "batch must be sorted"
    order = np.argsort(-counts, kind="stable")  # segment ids, width desc
    slot_w = np.maximum(counts[order[::NCORES][:S]], 1)  # width of rank 8k
    tiles, cols = _plan_tiles(slot_w)

    key = (cols, tuple(slot_w.tolist()))
    if key not in _compiled_cache:
        _compiled_cache[key] = _build_program(tiles, cols)
    nc = _compiled_cache[key]

    # column start / padded width of each slot
    slot_col = np.zeros(S, dtype=np.int64)
    slot_wt = np.zeros(S, dtype=np.int64)
    for k0, d, wt, col0 in tiles:
        for j in range(d):
            slot_col[k0 + j] = col0 + j * wt
        slot_wt[k0 : k0 + d] = wt

    starts = np.searchsorted(batch, np.arange(NSEG), side="left")
    ends = np.searchsorted(batch, np.arange(NSEG), side="right")

    bf = ml_dtypes.bfloat16
    in_maps = []
    for c in range(NCORES):
        segs = order[np.arange(S) * NCORES + c]  # this core's segment ids
        ccnt = counts[segs]
        src = np.full(cols, -1, dtype=np.int64)
        for k in range(S):
            s = segs[k]
            cnt = int(ccnt[k])
            c0 = slot_col[k]
            if cnt:
                src[c0 : c0 + cnt] = np.arange(starts[s], ends[s])
                # pads duplicate the first member (exact for max; sum fixed
                # on device via the npad correction)
                src[c0 + cnt : c0 + slot_wt[k]] = starts[s]
        real = src >= 0
        xTc = np.zeros((C, cols), dtype=bf)
        xTc[:, real] = x[src[real]].T.astype(bf)
        npadc = np.where(ccnt > 0, -(slot_wt - ccnt), 0).astype(np.float32)
        recipc = (1.0 / np.maximum(ccnt, 1.0)).astype(np.float32)
        in_maps.append(
            dict(
                xT=xTc,
                w1=np.ascontiguousarray(W1p.T).astype(bf),
                w2=np.ascontiguousarray(W2p.T).astype(bf),
                w3=np.ascontiguousarray(W3p.T).astype(bf),
                b1=np.ascontiguousarray(b1p[:, None]),
                b2=np.ascontiguousarray(b2p[:, None]),
                b3=np.ascontiguousarray(b3p[:, None]),
                npad=np.ascontiguousarray(
                    np.broadcast_to(npadc[None, :], (H, S))
                ),
                wsum=np.ascontiguousarray(Wop[:, 0:H].T),
                wmax=np.ascontiguousarray(Wop[:, H : 2 * H].T),
                wmean=np.ascontiguousarray(Wop[:, 2 * H : 3 * H].T),
                bo=np.ascontiguousarray(bop[None, :]),
                recip=np.ascontiguousarray(recipc.reshape(S // H, H).T),
            )
        )

    ncores_run = int(os.environ.get("KERNEL_NCORES", str(NCORES)))
    res = bass_utils.run_bass_kernel_spmd(
        nc,
        in_maps[:ncores_run],
        core_ids=list(range(ncores_run)),
        trace=bool(int(os.environ.get("KERNEL_TRACE", "0"))),
        tmpdir=os.environ.get("KERNEL_TRACE_DIR") or None,
    )
    kernel.last_results = res

    out_full = np.zeros((NSEG, O), dtype=np.float32)
    ranks = np.arange(S)
    for c in range(ncores_run):
        out_full[order[ranks * NCORES + c]] = res.results[c]["out"]
    # empty segments: agg == 0, so the output row is just the folded bias
    out_full[counts == 0] = bop
    return out_full
